# revision 1
# baseline (speedup 1.0000x reference)
"""Trainium2 Bass kernel for nn_BoundaryExtractionModule.

Data-parallel over batch: 8 samples -> 8 NeuronCores, one sample per core.

Per-core pipeline (channel-major layout [C, N] with C=64 on partitions):
  conv3x3(W_std)+depthwise-Laplacian   : 9 shift-matmuls per 512-col chunk
                                         (Laplacian folded into the taps on host)
  3-scale pooled non-local attention   : for each scale s in (4, 2, 1):
      A: row-max of logits  S = f^T f   (fp16 matmuls, DVE reduce_max)
      B: recompute S^T with the shift folded in via an augmented
         contraction row (K=65):  S'[m,q] = sum_k f_a[k,m] g_a[k,q]
         where f_a = [f; 1], g_a = [f; -rowmax]
      exp on ACT (PSUM -> fp16 SBUF)   : E^T tiles
      C: PV matmul with ones-column    : G = [f; 1] @ E^T  ->  G[64] = softmax denom
      D: out = G[0:64] * (1/G[64])     : gpsimd partition_broadcast + multiply
  bilinear x2/x4 upsample (half-pixel) : strided ops on edge-padded buffers
  residual add + DMA out.

The emission order interleaves the small scales and upsampling into scale-1's
superblock stream so every engine stays busy (Tile schedules greedily in
program order).
"""

import numpy as np

import concourse.bass as bass
import concourse.mybir as mybir
import concourse.tile as tile
from concourse import bacc
from concourse.bass_utils import run_bass_kernel_spmd
from concourse.masks import make_identity

dt = mybir.dt
AF = mybir.ActivationFunctionType
ALU = mybir.AluOpType
AX = mybir.AxisListType

C = 64
H = W = 64
N1 = H * W          # 4096
PAD = 66            # padded row length for conv
NCORES = 8

_cache = {}


def _v(ap, off, dims):
    """View of `ap` at free-offset `off` with free dims `dims` (keeps partition dim)."""
    return bass.AP(ap.tensor, ap.offset + off, [list(ap.ap[0])] + [list(d) for d in dims])


def _chunks(total, size):
    out = []
    off = 0
    while off < total:
        out.append((off, min(size, total - off)))
        off += size
    return out


def _build_nc():
    nc = bacc.Bacc(None, target_bir_lowering=False)
    xp_d = nc.dram_tensor("xp", [C, PAD * PAD], dt.float16, kind="ExternalInput")
    wt_d = nc.dram_tensor("wt", [C, 9 * C], dt.float16, kind="ExternalInput")
    out_d = nc.dram_tensor("out", [C, N1], dt.float32, kind="ExternalOutput")

    with tile.TileContext(nc) as tc:
        with (
            tc.tile_pool(name="sb", bufs=1) as sb,
            tc.tile_pool(name="ga", bufs=4) as ga_pool,
            tc.tile_pool(name="et", bufs=6) as et_pool,
            tc.tile_pool(name="dd", bufs=4) as dd_pool,
            tc.tile_pool(name="cm", bufs=18) as cm_pool,
            tc.tile_pool(name="aa", bufs=3, space="PSUM") as aa,
            tc.tile_pool(name="pp", bufs=2, space="PSUM") as pp,
            tc.tile_pool(name="gg", bufs=1, space="PSUM") as gg,
        ):
            # ---------------- inputs / constants ----------------
            xp16 = sb.tile([C, PAD * PAD], dt.float16)
            # split the input DMA so conv chunk 0 (rows 0..9) can start early
            nc.sync.dma_start(xp16[:, 0:10 * PAD], xp_d.ap()[:, 0:10 * PAD])
            nc.sync.dma_start(xp16[:, 10 * PAD:], xp_d.ap()[:, 10 * PAD:])
            wt16 = sb.tile([C, 9 * C], dt.float16)
            nc.sync.dma_start(wt16[:], wt_d.ap())

            ident = sb.tile([128, 128], dt.float16)
            make_identity(nc, ident[:])

            out_acc = sb.tile([C, N1], dt.float32)
            # residual init: out_acc = x  (from the padded fp16 input)
            nc.gpsimd.tensor_copy(out_acc[:], _v(xp16[:], PAD + 1, [[PAD, H], [1, W]]))

            f1a = sb.tile([C + 1, N1], dt.float16)
            fT1 = sb.tile([128, 32 * 65], dt.float16)
            nc.vector.memset(_v(fT1[:], C, [[65, 32]]), 1.0)
            nc.vector.memset(f1a[C:C + 1, :], 1.0)

            # ---------------- generic attention (per-superblock emitter) ----------------
            def build_fT(fa, NT, name):
                fT = sb.tile([128, NT * 65], dt.float16, tag=name)
                nc.vector.memset(_v(fT[:], C, [[65, NT]]), 1.0)
                for j in range(NT):
                    pt = pp.tile([128, C], dt.float16, tag="b")
                    nc.tensor.transpose(pt[:], fa[0:C, j * 128:(j + 1) * 128], ident[0:C, 0:C])
                    nc.scalar.copy(fT[:, j * 65:j * 65 + C], pt[:])
                return fT

            def _achunks(N):
                return _chunks(N, 512)

            def attn_A_start(fa, N, isb):
                q0 = isb * 512
                Q = min(512, N - q0)
                nsub = Q // 128
                achunks = _achunks(N)
                multi = len(achunks) > 1
                return dict(
                    fa=fa, N=N, isb=isb, q0=q0, Q=Q, nsub=nsub, achunks=achunks,
                    x1=[cm_pool.tile([128, 8], dt.float32, tag="x1", name=f"x1_{isb}_{s}")
                        for s in range(nsub)] if multi else None,
                    x2=[cm_pool.tile([128, 1], dt.float16, tag="x2", name=f"x2_{isb}_{s}")
                        for s in range(nsub)],
                )

            def attn_A_chunk(st, k):
                fa, q0 = st["fa"], st["q0"]
                off, ln = st["achunks"][k]
                for sub in range(st["nsub"]):
                    lhsA = fa[0:C, q0 + sub * 128: q0 + (sub + 1) * 128]
                    at = aa.tile([128, ln], dt.float32, tag="a")
                    for h0, hl in _chunks(ln, 512):
                        nc.tensor.matmul(at[:, h0:h0 + hl], lhsA,
                                         fa[0:C, off + h0:off + h0 + hl],
                                         start=True, stop=True)
                    if st["x1"] is None:
                        nc.vector.reduce_max(st["x2"][sub][:], at[:], axis=AX.X, negate=True)
                    else:
                        nc.vector.reduce_max(st["x1"][sub][:, k:k + 1], at[:], axis=AX.X)

            def attn_finish(st, fT, write_out, filler=()):
                fa, N, isb = st["fa"], st["N"], st["isb"]
                q0, Q, nsub = st["q0"], st["Q"], st["nsub"]
                NT = N // 128
                nch = len(st["achunks"])
                ga = ga_pool.tile([C + 1, Q], dt.float16, tag="ga")
                nc.vector.tensor_copy(ga[0:C, :], fa[0:C, q0:q0 + Q])
                for sub in range(nsub):
                    x2 = st["x2"][sub]
                    if st["x1"] is not None:
                        nc.vector.reduce_max(x2[:], st["x1"][sub][:, 0:nch],
                                             axis=AX.X, negate=True)
                    # PE-transpose -max [128,1] -> [1,128] into the g_a bias row
                    pt = aa.tile([1, 128], dt.float16, tag="a")
                    nc.tensor.transpose(pt[:], x2[:], ident[:])
                    nc.vector.tensor_copy(ga[C:C + 1, sub * 128:(sub + 1) * 128], pt[:])
                # --- B + exp + C (filler thunks keep PE fed while exp runs) ---
                G = gg.tile([C + 1, Q], dt.float32, tag="g")
                mtiles = list(range(NT))
                groups = [mtiles[i:i + 2] for i in range(0, NT, 2)]
                filler = list(filler)
                fill_at = {int(i * len(groups) / len(filler)): i for i in range(len(filler))} if filler else {}
                for gi, grp in enumerate(groups):
                    if gi in fill_at:
                        filler[fill_at[gi]]()
                    bt = pp.tile([128, 512 * len(grp)], dt.float32, tag="b")
                    et = et_pool.tile([128, 512 * len(grp)], dt.float16, tag="et")
                    for jj, j in enumerate(grp):
                        nc.tensor.matmul(bt[:, jj * 512: jj * 512 + Q],
                                         fa[:, j * 128:(j + 1) * 128], ga[:],
                                         start=True, stop=True)
                    if Q == 512:
                        nc.scalar.activation(et[:], bt[:], AF.Exp)
                    else:
                        for jj in range(len(grp)):
                            nc.scalar.activation(et[:, jj * 512:jj * 512 + Q],
                                                 bt[:, jj * 512:jj * 512 + Q], AF.Exp)
                    for jj, j in enumerate(grp):
                        nc.tensor.matmul(G[:], fT[:, j * 65:(j + 1) * 65],
                                         et[:, jj * 512:jj * 512 + Q],
                                         start=(gi == 0 and jj == 0),
                                         stop=(j == NT - 1))
                # --- D: normalize ---
                Gs = dd_pool.tile([C + 1, 512], dt.float32, tag="gs")
                nc.scalar.copy(Gs[:, 0:Q], G[:])
                linv = dd_pool.tile([1, 512], dt.float32, tag="linv")
                nc.vector.reciprocal(linv[:, 0:Q], Gs[C:C + 1, 0:Q])
                lrep = dd_pool.tile([C, 512], dt.float32, tag="lrep")
                nc.gpsimd.partition_broadcast(lrep[:, 0:Q], linv[0:1, 0:Q])
                write_out(isb, q0, Q, Gs, lrep)

            def w1(isb, q0, Q, Gs, lrep):
                eng = nc.gpsimd
                tmp = dd_pool.tile([C, 512], dt.float32, tag="tmp")
                eng.tensor_tensor(tmp[:, 0:Q], Gs[0:C, 0:Q], lrep[:, 0:Q], op=ALU.mult)
                eng.tensor_tensor(out_acc[:, q0:q0 + Q], out_acc[:, q0:q0 + Q],
                                  tmp[:, 0:Q], op=ALU.add)

            att2p = sb.tile([C, 34 * 34], dt.float32)   # scale-2 attn out, 1-px padded
            att4p = sb.tile([C, 18 * 18], dt.float32)   # scale-4 attn out, 1-px padded
            up_acc = sb.tile([C, N1], dt.float32)       # upsampled x2+x4 sum

            def w2(isb, q0, Q, Gs, lrep):
                r0 = isb * 16
                view = _v(att2p[:], (1 + r0) * 34 + 1, [[34, 16], [1, 32]])
                nc.gpsimd.tensor_tensor(view, Gs[0:C, 0:Q], lrep[:, 0:Q], op=ALU.mult)

            def w4(isb, q0, Q, Gs, lrep):
                view = _v(att4p[:], 18 + 1, [[18, 16], [1, 16]])
                nc.gpsimd.tensor_tensor(view, Gs[0:C, 0:Q], lrep[:, 0:Q], op=ALU.mult)

            # ---------------- pool emitters (gpsimd) ----------------
            f2raw = sb.tile([C, 1024], dt.float32)
            f2a = sb.tile([C + 1, 1024], dt.float16)
            f4a = sb.tile([C + 1, 256], dt.float16)

            def emit_pools2():
                f1 = f1a[0:C, :]
                t2w = sb.tile([C, 2048], dt.float32)
                nc.gpsimd.tensor_tensor(t2w[:], _v(f1, 0, [[2, 2048]]), _v(f1, 1, [[2, 2048]]), op=ALU.add)
                nc.gpsimd.tensor_tensor(f2raw[:], _v(t2w[:], 0, [[64, 32], [1, 32]]),
                                        _v(t2w[:], 32, [[64, 32], [1, 32]]), op=ALU.add)
                nc.gpsimd.tensor_scalar_mul(f2a[0:C, :], f2raw[:], 0.25)
                nc.gpsimd.memset(f2a[C:C + 1, :], 1.0)

            def emit_pools4():
                t4w = sb.tile([C, 512], dt.float32)
                nc.gpsimd.tensor_tensor(t4w[:], _v(f2raw[:], 0, [[2, 512]]), _v(f2raw[:], 1, [[2, 512]]), op=ALU.add)
                f4raw = sb.tile([C, 256], dt.float32)
                nc.gpsimd.tensor_tensor(f4raw[:], _v(t4w[:], 0, [[32, 16], [1, 16]]),
                                        _v(t4w[:], 16, [[32, 16], [1, 16]]), op=ALU.add)
                nc.gpsimd.tensor_scalar_mul(f4a[0:C, :], f4raw[:], 1.0 / 16.0)
                nc.gpsimd.memset(f4a[C:C + 1, :], 1.0)

            # ---------------- upsample emitters ----------------
            def emit_up4():
                p4 = att4p[:]
                ups = sb.tile([C, 256], dt.float32, tag="ups4")
                # edge replication (cols then rows so corners fill correctly)
                nc.gpsimd.tensor_copy(_v(p4, 18, [[18, 16]]), _v(p4, 19, [[18, 16]]))
                nc.gpsimd.tensor_copy(_v(p4, 18 + 17, [[18, 16]]), _v(p4, 18 + 16, [[18, 16]]))
                nc.gpsimd.tensor_copy(_v(p4, 0, [[1, 18]]), _v(p4, 18, [[1, 18]]))
                nc.gpsimd.tensor_copy(_v(p4, 17 * 18, [[1, 18]]), _v(p4, 16 * 18, [[1, 18]]))
                # W-stage: t4u rows 1..16 (padded layout [C, 18, 64]) on gpsimd
                t4u = sb.tile([C, 18 * 64], dt.float32)
                pre58 = sb.tile([C, 256], dt.float32)   # 0.625 * center
                pre78 = sb.tile([C, 256], dt.float32)   # 0.875 * center
                ctr = _v(p4, 18 + 1, [[18, 16], [1, 16]])
                nc.gpsimd.tensor_scalar_mul(pre58[:], ctr, 0.625)
                nc.gpsimd.tensor_scalar_mul(pre78[:], ctr, 0.875)
                lft = _v(p4, 18 + 0, [[18, 16], [1, 16]])
                rgt = _v(p4, 18 + 2, [[18, 16], [1, 16]])
                for p, (nb, a, pre) in enumerate([(lft, 0.375, pre58), (lft, 0.125, pre78),
                                                  (rgt, 0.125, pre78), (rgt, 0.375, pre58)]):
                    outv = _v(t4u[:], 64 + p, [[64, 16], [4, 16]])
                    nc.gpsimd.tensor_scalar_mul(ups[:], nb, a)
                    nc.gpsimd.tensor_tensor(outv, ups[:], pre[:], op=ALU.add)
                nc.gpsimd.tensor_copy(_v(t4u[:], 0, [[1, 64]]), _v(t4u[:], 64, [[1, 64]]))
                nc.gpsimd.tensor_copy(_v(t4u[:], 17 * 64, [[1, 64]]), _v(t4u[:], 16 * 64, [[1, 64]]))
                # H-stage into up_acc (rows I = 4r+p): first op writes, second accumulates
                u4s = sb.tile([C, 1024], dt.float32)
                for p, (o1, a1, o2, a2) in enumerate([(0, 0.375, 64, 0.625), (0, 0.125, 64, 0.875),
                                                      (64, 0.875, 128, 0.125), (64, 0.625, 128, 0.375)]):
                    outv = _v(up_acc[:], p * 64, [[256, 16], [1, 64]])
                    nc.gpsimd.tensor_scalar_mul(outv, _v(t4u[:], o1, [[64, 16], [1, 64]]), a1)
                    nc.gpsimd.tensor_scalar_mul(u4s[:], _v(t4u[:], o2, [[64, 16], [1, 64]]), a2)
                    nc.gpsimd.tensor_tensor(outv, outv, u4s[:], op=ALU.add)

            def emit_up2():
                p2 = att2p[:]
                ups = sb.tile([C, 1024], dt.float32, tag="ups2")
                nc.gpsimd.tensor_copy(_v(p2, 34, [[34, 32]]), _v(p2, 35, [[34, 32]]))
                nc.gpsimd.tensor_copy(_v(p2, 34 + 33, [[34, 32]]), _v(p2, 34 + 32, [[34, 32]]))
                nc.gpsimd.tensor_copy(_v(p2, 0, [[1, 34]]), _v(p2, 34, [[1, 34]]))
                nc.gpsimd.tensor_copy(_v(p2, 33 * 34, [[1, 34]]), _v(p2, 32 * 34, [[1, 34]]))
                t2u = sb.tile([C, 34 * 64], dt.float32)
                pre34 = sb.tile([C, 1024], dt.float32)  # 0.75 * center
                ctr2 = _v(p2, 34 + 1, [[34, 32], [1, 32]])
                nc.gpsimd.tensor_scalar_mul(pre34[:], ctr2, 0.75)
                lft2 = _v(p2, 34 + 0, [[34, 32], [1, 32]])
                rgt2 = _v(p2, 34 + 2, [[34, 32], [1, 32]])
                for p, nb in enumerate([lft2, rgt2]):
                    outv = _v(t2u[:], 64 + p, [[64, 32], [2, 32]])
                    nc.gpsimd.tensor_scalar_mul(ups[:], nb, 0.25)
                    nc.gpsimd.tensor_tensor(outv, ups[:], pre34[:], op=ALU.add)
                nc.gpsimd.tensor_copy(_v(t2u[:], 0, [[1, 64]]), _v(t2u[:], 64, [[1, 64]]))
                nc.gpsimd.tensor_copy(_v(t2u[:], 33 * 64, [[1, 64]]), _v(t2u[:], 32 * 64, [[1, 64]]))
                u2s = sb.tile([C, 2048], dt.float32)
                for p, (o1, a1, o2, a2) in enumerate([(0, 0.25, 64, 0.75), (64, 0.75, 128, 0.25)]):
                    outv = _v(up_acc[:], p * 64, [[128, 32], [1, 64]])
                    for off, coef in ((o1, a1), (o2, a2)):
                        nc.gpsimd.tensor_scalar_mul(u2s[:], _v(t2u[:], off, [[64, 32], [1, 64]]), coef)
                        nc.gpsimd.tensor_tensor(outv, outv, u2s[:], op=ALU.add)

            def attn_sb(fa, fT, N, isb, write_out):
                st = attn_A_start(fa, N, isb)
                for k in range(len(st["achunks"])):
                    attn_A_chunk(st, k)
                attn_finish(st, fT, write_out)

            # ---------------- master schedule ----------------
            # conv chunks interleaved with fT1 build and sb0/sb1's A-pass
            # (A-chunk k only needs conv chunk k evicted).
            st0 = attn_A_start(f1a, N1, 0)
            st1 = attn_A_start(f1a, N1, 1)
            # A-chunk (st, k) becomes runnable once conv has evicted its columns
            asched = {0: [(st0, 0)], 1: [(st1, 0), (st0, 1)], 2: [(st1, 1), (st0, 2)],
                      3: [(st1, 2), (st0, 3)], 4: [(st1, 3), (st0, 4)],
                      5: [(st1, 4), (st0, 5)], 6: [(st1, 5), (st0, 6)],
                      7: [(st1, 6), (st0, 7)]}
            for r in range(8):
                cp = pp.tile([C, 512], dt.float32, tag="b")
                for tap in range(9):
                    dy, dx = divmod(tap, 3)
                    rhs = _v(xp16[:], (8 * r + dy) * PAD + dx, [[PAD, 8], [1, W]])
                    nc.tensor.matmul(cp[:], wt16[:, tap * C:(tap + 1) * C], rhs,
                                     start=(tap == 0), stop=(tap == 8))
                nc.scalar.copy(f1a[0:C, r * 512:(r + 1) * 512], cp[:])
                for st, k in asched.get(r, []):
                    attn_A_chunk(st, k)
                for j in range(4 * r, 4 * r + 4):
                    pt = pp.tile([128, C], dt.float16, tag="b")
                    nc.tensor.transpose(pt[:], f1a[0:C, j * 128:(j + 1) * 128], ident[0:C, 0:C])
                    nc.scalar.copy(fT1[:, j * 65:j * 65 + C], pt[:])
            attn_A_chunk(st1, 7)

            def fill_chunks(st):
                return [(lambda st=st, k=k: attn_A_chunk(st, k))
                        for k in range(len(st["achunks"]))]

            emit_pools2()
            st2 = attn_A_start(f1a, N1, 2)
            attn_finish(st0, fT1, w1, filler=fill_chunks(st2))
            st3 = attn_A_start(f1a, N1, 3)
            attn_finish(st1, fT1, w1, filler=fill_chunks(st3))
            fT2 = build_fT(f2a, 8, "fT2")
            st4 = attn_A_start(f1a, N1, 4)
            attn_finish(st2, fT1, w1, filler=fill_chunks(st4))
            attn_sb(f2a, fT2, 1024, 0, w2)
            st5 = attn_A_start(f1a, N1, 5)
            attn_finish(st3, fT1, w1, filler=fill_chunks(st5))
            attn_sb(f2a, fT2, 1024, 1, w2)
            emit_pools4()
            st6 = attn_A_start(f1a, N1, 6)
            attn_finish(st4, fT1, w1, filler=fill_chunks(st6))
            fT4 = build_fT(f4a, 2, "fT4")
            attn_sb(f4a, fT4, 256, 0, w4)
            emit_up4()
            st7 = attn_A_start(f1a, N1, 7)
            attn_finish(st5, fT1, w1, filler=fill_chunks(st7))
            emit_up2()
            attn_finish(st6, fT1, w1)
            # last superblock: the final up_acc add + most of the output DMA
            # overlap its B/C window (DVE/DMA are otherwise idle there).
            nc.vector.tensor_tensor(out_acc[:, 0:3584], out_acc[:, 0:3584],
                                    up_acc[:, 0:3584], op=ALU.add)
            nc.sync.dma_start(out_d.ap()[:, 0:3584], out_acc[:, 0:3584])
            attn_finish(st7, fT1, w1)
            nc.gpsimd.tensor_tensor(out_acc[:, 3584:N1], out_acc[:, 3584:N1],
                                    up_acc[:, 3584:N1], op=ALU.add)
            nc.sync.dma_start(out_d.ap()[:, 3584:N1], out_acc[:, 3584:N1])

    nc.compile()
    return nc


def _prep_inputs(x, W_std):
    lap = np.array([[0., 1., 0.], [1., -4., 1.], [0., 1., 0.]], dtype=np.float32)
    Wl = W_std.astype(np.float32) + lap[None, None] * np.eye(C, dtype=np.float32)[:, :, None, None]
    wt = np.ascontiguousarray(Wl.transpose(1, 2, 3, 0).reshape(C, 9 * C)).astype(np.float16)
    B = x.shape[0]
    xps = np.zeros((B, C, PAD, PAD), dtype=np.float16)
    xps[:, :, 1:H + 1, 1:W + 1] = x.astype(np.float16)
    return xps.reshape(B, C, PAD * PAD), wt


def _run(x, W_std, trace=False):
    x = np.asarray(x)
    W_std = np.asarray(W_std)
    xps, wt = _prep_inputs(x, W_std)
    if "nc" not in _cache:
        _cache["nc"] = _build_nc()
    nc = _cache["nc"]
    in_maps = [{"xp": np.ascontiguousarray(xps[i]), "wt": wt} for i in range(x.shape[0])]
    ncores = min(NCORES, x.shape[0])
    res = run_bass_kernel_spmd(nc, in_maps, core_ids=list(range(ncores)), trace=trace)
    out = np.stack([res.results[i]["out"].reshape(C, H, W) for i in range(x.shape[0])])
    return out.astype(np.float32), res


def kernel(x, W_std):
    out, _ = _run(x, W_std, trace=False)
    return out



# revision 11
# speedup vs baseline: 1.0976x; 1.0976x over previous
"""Trainium2 Bass kernel for nn_BoundaryExtractionModule.

Data-parallel over batch: 8 samples -> 8 NeuronCores, one sample per core.

Per-core pipeline (channel-major layout [C, N] with C=64 on partitions):
  conv3x3(W_std)+depthwise-Laplacian  : tap-packed 5 matmuls per 512-col chunk
                                        (two shifted input copies stacked on
                                        128 partitions give K=128 tap pairs)
  3-scale non-local attention         : for each scale s in (4, 2, 1):
      A: approximate row-max c of the logits S = f^T f.  Scale 1 maxes the
         even columns (stride-2 A-matmuls, half cost) plus the diagonal
         |f_q|^2 (from fT squares); scales 2/4 max exactly.  A sigmoid
         in place of exp makes the softmax immune to the underestimate:
         sigmoid(x-d) = e^(x-d) for x <= c (0.3% at d=6) and saturates at 1
         for the rare rows whose true max was missed, so no overflow and
         the missed peak still dominates.  The shift d cancels in the ratio.
      B: recompute S^T with -(c+6) folded in via an augmented contraction
         row (K=65):  S'[m,q] = sum_k f_a[k,m] g_a[k,q],  g_a = [f; -(c+6)]
      sigmoid on ACT (PSUM -> fp16 SBUF) : E^T tiles
      C: PV matmul (fp16)               : G = [f;1] @ E^T, ones-column gives
                                          the softmax denominator
      D: normalize: reciprocal of the denominator row, gpsimd
         partition-broadcast, multiply, accumulate.
  bilinear x2/x4 upsample (half-pixel): stt-fused strided ops on DVE/Pool
  residual add + DMA out.
"""

import numpy as np

import concourse.bass as bass
import concourse.mybir as mybir
import concourse.tile as tile
from concourse import bacc
from concourse.bass_utils import run_bass_kernel_spmd
from concourse.masks import make_identity

dt = mybir.dt
AF = mybir.ActivationFunctionType
ALU = mybir.AluOpType
AX = mybir.AxisListType

C = 64
H = W = 64
N1 = H * W          # 4096
PAD = 66            # padded row length for conv
NCORES = 8
DSH = 6.0           # sigmoid shift

_cache = {}


def _v(ap, off, dims):
    """View of `ap` at free-offset `off` with free dims `dims` (keeps partition dim)."""
    return bass.AP(ap.tensor, ap.offset + off, [list(ap.ap[0])] + [list(d) for d in dims])


def _build_nc():
    nc = bacc.Bacc(None, target_bir_lowering=False)
    xp_d = nc.dram_tensor("xp", [C, PAD * PAD], dt.float16, kind="ExternalInput")
    wt_d = nc.dram_tensor("wt", [128, 5 * C], dt.float16, kind="ExternalInput")
    out_d = nc.dram_tensor("out", [C, N1], dt.float32, kind="ExternalOutput")

    with tile.TileContext(nc) as tc:
        with (
            tc.tile_pool(name="sb", bufs=1) as sb,
            tc.tile_pool(name="ga", bufs=4) as ga_pool,
            tc.tile_pool(name="et", bufs=6) as et_pool,
            tc.tile_pool(name="dd", bufs=4) as dd_pool,
            tc.tile_pool(name="aa", bufs=3, space="PSUM") as aa,
            tc.tile_pool(name="pp", bufs=2, space="PSUM") as pp,
            tc.tile_pool(name="gg", bufs=1, space="PSUM") as gg,
        ):
            # ---------------- inputs ----------------
            # XA: partitions 0:64 = x, 64:128 = x shifted +1 col (dx tap pairs)
            # XB: partitions 0:64 = x, 64:128 = x shifted +1 row
            XA = sb.tile([128, PAD * PAD], dt.float16)
            XB = sb.tile([128, PAD * PAD], dt.float16)
            head = 10 * PAD
            nc.sync.dma_start(XA[0:C, 0:head], xp_d.ap()[:, 0:head])
            nc.sync.dma_start(XA[C:128, 0:head], xp_d.ap()[:, 1:head + 1])
            nc.sync.dma_start(XB[0:C, 0:head], xp_d.ap()[:, 0:head])
            nc.sync.dma_start(XB[C:128, 0:head], xp_d.ap()[:, PAD:head + PAD])
            nc.sync.dma_start(XA[0:C, head:PAD * PAD], xp_d.ap()[:, head:PAD * PAD])
            nc.sync.dma_start(XA[C:128, head:PAD * PAD - 1], xp_d.ap()[:, head + 1:PAD * PAD])
            nc.sync.dma_start(XB[0:C, head:PAD * PAD], xp_d.ap()[:, head:PAD * PAD])
            nc.sync.dma_start(XB[C:128, head:PAD * PAD - PAD], xp_d.ap()[:, head + PAD:PAD * PAD])
            wt16 = sb.tile([128, 5 * C], dt.float16)
            nc.sync.dma_start(wt16[:], wt_d.ap())

            ident = sb.tile([128, 128], dt.float16)
            make_identity(nc, ident[:])

            out_acc = sb.tile([C, N1], dt.float32)
            # residual init: out_acc = x  (from the padded fp16 input)
            nc.gpsimd.tensor_copy(out_acc[:], _v(XA[0:C, :], PAD + 1, [[PAD, H], [1, W]]))

            # features (fp16) with augmented ones row
            f1a = sb.tile([C + 1, N1], dt.float16)
            nc.vector.memset(f1a[C:C + 1, :], 1.0)

            # transposed features (C-pass lhsT, 65-row layout with ones col)
            fT1 = sb.tile([128, 32 * 65], dt.float16)
            fT2 = sb.tile([128, 8 * 65], dt.float16)
            fT4 = sb.tile([128, 2 * 65], dt.float16)
            for t, nt in ((fT1, 32), (fT2, 8), (fT4, 2)):
                nc.vector.memset(_v(t[:], C, [[65, nt], [1, 1]]), 1.0)
            dsq = sb.tile([128, C], dt.float16)   # fT square scratch

            def ft_convert(fa, fT, j0, nj, diag_into=None):
                """PE-transpose fa j-tiles into fT; optionally also compute
                the diagonal |f_q|^2 of tile j into diag_into(j) (an x1 col)."""
                for j in range(j0, j0 + nj, 2):
                    take = min(2, j0 + nj - j)
                    pt = aa.tile([128, 128], dt.float16, tag="a")
                    for u in range(take):
                        nc.tensor.transpose(pt[:, u * C:(u + 1) * C],
                                            fa[0:C, (j + u) * 128:(j + u + 1) * 128],
                                            ident[0:C, 0:C])
                    nc.vector.tensor_copy(_v(fT[:], j * 65, [[65, take], [1, C]]),
                                          _v(pt[:], 0, [[C, take], [1, C]]))
                    if diag_into is not None:
                        for u in range(take):
                            ftv = fT[:, (j + u) * 65:(j + u) * 65 + C]
                            nc.vector.tensor_tensor(dsq[:], ftv, ftv, op=ALU.mult)
                            nc.vector.reduce_sum(diag_into(j + u), dsq[:], axis=AX.X)

            # ---------------- A-pass: row-max candidates ----------------
            # scale 1: stride-2 even columns (4 tiles/sub) + diag col in x1.
            # scales 2/4: exact.  x2 = -(max over x1 cols) - DSH.
            def attn_A_start(fa, N, isb, label):
                q0 = isb * 512
                Q = min(512, N - q0)
                nsub = Q // 128
                st = dict(fa=fa, N=N, isb=isb, q0=q0, Q=Q, nsub=nsub,
                          nunits=4 if N > 1024 else (2 if N == 1024 else 1),
                          x1=[sb.tile([128, 6], dt.float32, name=f"x1_{label}_{isb}_{s}")
                              for s in range(nsub)],
                          x2=[sb.tile([128, 1], dt.float16, name=f"x2_{label}_{isb}_{s}")
                              for s in range(nsub)])
                return st

            def attn_A_unit(st, u):
                fa, q0, N = st["fa"], st["q0"], st["N"]
                for sub in range(st["nsub"]):
                    lhsA = fa[0:C, q0 + sub * 128: q0 + (sub + 1) * 128]
                    at = aa.tile([128, 512], dt.float32, tag="a")
                    if N > 1024:
                        # even columns of original chunk pair (2u, 2u+1)
                        rhs = _v(fa[0:C, :], u * 1024, [[2, 512]])
                        nc.tensor.matmul(at[:], lhsA, rhs, start=True, stop=True)
                        nc.vector.reduce_max(st["x1"][sub][:, u:u + 1], at[:],
                                             axis=AX.X)
                    else:
                        ln = min(512, N - u * 512)
                        nc.tensor.matmul(at[:, 0:ln], lhsA,
                                         fa[0:C, u * 512:u * 512 + ln],
                                         start=True, stop=True)
                        nc.vector.reduce_max(st["x1"][sub][:, u:u + 1], at[:, 0:ln],
                                             axis=AX.X)

            def attn_A_final(st):
                ncol = st["nunits"] + (1 if st["N"] > 1024 else 0)  # + diag col
                for sub in range(st["nsub"]):
                    x1 = st["x1"][sub]
                    m = dd_pool.tile([128, 1], dt.float32, tag="m")
                    nc.vector.reduce_max(m[:], x1[:, 0:ncol], axis=AX.X)
                    nc.vector.tensor_scalar(st["x2"][sub][:], m[:], -1.0, -DSH,
                                            op0=ALU.mult, op1=ALU.add)

            # ---------------- B + sigmoid + C + normalize ----------------
            def attn_finish(st, fT, write_out, filler=()):
                fa, N, isb = st["fa"], st["N"], st["isb"]
                q0, Q, nsub = st["q0"], st["Q"], st["nsub"]
                NT = N // 128
                attn_A_final(st)
                ga = ga_pool.tile([C + 1, 512], dt.float16, tag="ga")
                nc.vector.tensor_copy(ga[0:C, 0:Q], fa[0:C, q0:q0 + Q])
                for sub in range(nsub):
                    # PE-transpose -(c+6) [128,1] -> [1,128] into the bias row
                    pt = aa.tile([1, 128], dt.float16, tag="a")
                    nc.tensor.transpose(pt[:], st["x2"][sub][:], ident[:])
                    nc.vector.tensor_copy(ga[C:C + 1, sub * 128:(sub + 1) * 128], pt[:])
                G = gg.tile([C + 1, 512], dt.float32, tag="g")
                groups = [list(range(g, min(g + 2, NT))) for g in range(0, NT, 2)]
                filler = list(filler)
                fill_at = {int(i * len(groups) / len(filler)): i for i in range(len(filler))} if filler else {}
                for gi, grp in enumerate(groups):
                    if gi in fill_at:
                        filler[fill_at[gi]]()
                    bt = pp.tile([128, 1024], dt.float32, tag="b")
                    et = et_pool.tile([128, 1024], dt.float16, tag="et")
                    for jj, j in enumerate(grp):
                        nc.tensor.matmul(bt[:, jj * 512: jj * 512 + Q],
                                         fa[:, j * 128:(j + 1) * 128], ga[:, 0:Q],
                                         start=True, stop=True)
                    if Q == 512:
                        nc.scalar.activation(et[:], bt[:], AF.Sigmoid)
                    else:
                        for jj in range(len(grp)):
                            nc.scalar.activation(et[:, jj * 512:jj * 512 + Q],
                                                 bt[:, jj * 512:jj * 512 + Q],
                                                 AF.Sigmoid)
                    for jj, j in enumerate(grp):
                        nc.tensor.matmul(G[:, 0:Q], fT[:, j * 65:(j + 1) * 65],
                                         et[:, jj * 512:jj * 512 + Q],
                                         start=(gi == 0 and jj == 0),
                                         stop=(j == NT - 1))
                # --- D: normalize ---
                Gs = dd_pool.tile([C + 1, 512], dt.float32, tag="gs")
                nc.vector.tensor_copy(Gs[:, 0:Q], G[:, 0:Q])
                linv = dd_pool.tile([1, 512], dt.float32, tag="linv")
                nc.vector.reciprocal(linv[:, 0:Q], Gs[C:C + 1, 0:Q])
                lrep = dd_pool.tile([C, 512], dt.float32, tag="lrep")
                nc.gpsimd.partition_broadcast(lrep[:, 0:Q], linv[0:1, 0:Q])
                write_out(isb, q0, Q, Gs, lrep)

            def w1(isb, q0, Q, Gs, lrep):
                tmp = dd_pool.tile([C, 512], dt.float32, tag="tmp")
                nc.gpsimd.tensor_tensor(tmp[:, 0:Q], Gs[0:C, 0:Q], lrep[:, 0:Q], op=ALU.mult)
                nc.gpsimd.tensor_tensor(out_acc[:, q0:q0 + Q], out_acc[:, q0:q0 + Q],
                                        tmp[:, 0:Q], op=ALU.add)

            att2p = sb.tile([C, 34 * 34], dt.float32)   # scale-2 attn out, 1-px padded
            att4p = sb.tile([C, 18 * 18], dt.float32)   # scale-4 attn out, 1-px padded
            up_acc = sb.tile([C, N1], dt.float32)       # upsampled x2+x4 sum

            def w2(isb, q0, Q, Gs, lrep):
                r0 = isb * 16
                view = _v(att2p[:], (1 + r0) * 34 + 1, [[34, 16], [1, 32]])
                nc.gpsimd.tensor_tensor(view, Gs[0:C, 0:Q], lrep[:, 0:Q], op=ALU.mult)

            def w4(isb, q0, Q, Gs, lrep):
                view = _v(att4p[:], 18 + 1, [[18, 16], [1, 16]])
                nc.gpsimd.tensor_tensor(view, Gs[0:C, 0:Q], lrep[:, 0:Q], op=ALU.mult)

            # ---------------- pool emitters (DVE, fp16) ----------------
            f2raw = sb.tile([C, 1024], dt.float16)
            f2a = sb.tile([C + 1, 1024], dt.float16)
            f4a = sb.tile([C + 1, 256], dt.float16)

            def emit_pools2():
                f1 = f1a[0:C, :]
                t2w = sb.tile([C, 2048], dt.float16)
                nc.vector.tensor_tensor(t2w[:], _v(f1, 0, [[2, 2048]]), _v(f1, 1, [[2, 2048]]), op=ALU.add)
                nc.vector.tensor_tensor(f2raw[:], _v(t2w[:], 0, [[64, 32], [1, 32]]),
                                        _v(t2w[:], 32, [[64, 32], [1, 32]]), op=ALU.add)
                nc.vector.tensor_scalar_mul(f2a[0:C, :], f2raw[:], 0.25)
                nc.vector.memset(f2a[C:C + 1, :], 1.0)

            def emit_pools4():
                t4w = sb.tile([C, 512], dt.float16)
                nc.vector.tensor_tensor(t4w[:], _v(f2raw[:], 0, [[2, 512]]), _v(f2raw[:], 1, [[2, 512]]), op=ALU.add)
                f4raw = sb.tile([C, 256], dt.float16)
                nc.vector.tensor_tensor(f4raw[:], _v(t4w[:], 0, [[32, 16], [1, 16]]),
                                        _v(t4w[:], 16, [[32, 16], [1, 16]]), op=ALU.add)
                nc.vector.tensor_scalar_mul(f4a[0:C, :], f4raw[:], 1.0 / 16.0)
                nc.vector.memset(f4a[C:C + 1, :], 1.0)

            # ---------------- upsample emitters (stt-fused) ----------------
            ups_scr = sb.tile([C, 2048], dt.float32)   # upsample scratch

            def fma(eng, outv, in0, a, in1):
                """outv = in0*a + in1 (stt on DVE; mul+add pair on Pool)."""
                if eng is nc.vector:
                    eng.scalar_tensor_tensor(outv, in0, a, in1,
                                             op0=ALU.mult, op1=ALU.add)
                else:
                    sz = 1
                    for _, n in (in0.ap[1:] if len(in0.ap) > 1 else []):
                        sz *= n
                    scr = _v(ups_scr[:], 0, [[1, sz]])
                    eng.tensor_scalar_mul(scr, in0, a)
                    eng.tensor_tensor(outv, scr, in1, op=ALU.add)

            def emit_up4():
                p4 = att4p[:]
                nc.gpsimd.tensor_copy(_v(p4, 18, [[18, 16]]), _v(p4, 19, [[18, 16]]))
                nc.gpsimd.tensor_copy(_v(p4, 18 + 17, [[18, 16]]), _v(p4, 18 + 16, [[18, 16]]))
                nc.gpsimd.tensor_copy(_v(p4, 0, [[1, 18]]), _v(p4, 18, [[1, 18]]))
                nc.gpsimd.tensor_copy(_v(p4, 17 * 18, [[1, 18]]), _v(p4, 16 * 18, [[1, 18]]))
                t4u = sb.tile([C, 18 * 64], dt.float32)
                pre58 = sb.tile([C, 256], dt.float32)   # 0.625 * center
                pre78 = sb.tile([C, 256], dt.float32)   # 0.875 * center
                ctr = _v(p4, 18 + 1, [[18, 16], [1, 16]])
                nc.vector.tensor_scalar_mul(pre58[:], ctr, 0.625)
                nc.vector.tensor_scalar_mul(pre78[:], ctr, 0.875)
                lft = _v(p4, 18 + 0, [[18, 16], [1, 16]])
                rgt = _v(p4, 18 + 2, [[18, 16], [1, 16]])
                for p, (nb, a, pre) in enumerate([(lft, 0.375, pre58), (lft, 0.125, pre78),
                                                  (rgt, 0.125, pre78), (rgt, 0.375, pre58)]):
                    outv = _v(t4u[:], 64 + p, [[64, 16], [4, 16]])
                    eng = nc.vector if p % 2 == 0 else nc.gpsimd
                    fma(eng, outv, nb, a, pre[:])
                nc.gpsimd.tensor_copy(_v(t4u[:], 0, [[1, 64]]), _v(t4u[:], 64, [[1, 64]]))
                nc.gpsimd.tensor_copy(_v(t4u[:], 17 * 64, [[1, 64]]), _v(t4u[:], 16 * 64, [[1, 64]]))
                for p, (o1, a1, o2, a2) in enumerate([(0, 0.375, 64, 0.625), (0, 0.125, 64, 0.875),
                                                      (64, 0.875, 128, 0.125), (64, 0.625, 128, 0.375)]):
                    outv = _v(up_acc[:], p * 64, [[256, 16], [1, 64]])
                    eng = nc.vector if p % 2 == 0 else nc.gpsimd
                    eng.tensor_scalar_mul(outv, _v(t4u[:], o1, [[64, 16], [1, 64]]), a1)
                    fma(eng, outv, _v(t4u[:], o2, [[64, 16], [1, 64]]), a2, outv)

            def emit_up2():
                p2 = att2p[:]
                nc.gpsimd.tensor_copy(_v(p2, 34, [[34, 32]]), _v(p2, 35, [[34, 32]]))
                nc.gpsimd.tensor_copy(_v(p2, 34 + 33, [[34, 32]]), _v(p2, 34 + 32, [[34, 32]]))
                nc.gpsimd.tensor_copy(_v(p2, 0, [[1, 34]]), _v(p2, 34, [[1, 34]]))
                nc.gpsimd.tensor_copy(_v(p2, 33 * 34, [[1, 34]]), _v(p2, 32 * 34, [[1, 34]]))
                t2u = sb.tile([C, 34 * 64], dt.float32)
                pre34 = sb.tile([C, 1024], dt.float32)  # 0.75 * center
                ctr2 = _v(p2, 34 + 1, [[34, 32], [1, 32]])
                nc.vector.tensor_scalar_mul(pre34[:], ctr2, 0.75)
                lft2 = _v(p2, 34 + 0, [[34, 32], [1, 32]])
                rgt2 = _v(p2, 34 + 2, [[34, 32], [1, 32]])
                for p, nb in enumerate([lft2, rgt2]):
                    outv = _v(t2u[:], 64 + p, [[64, 32], [2, 32]])
                    eng = nc.vector if p == 0 else nc.gpsimd
                    fma(eng, outv, nb, 0.25, pre34[:])
                nc.gpsimd.tensor_copy(_v(t2u[:], 0, [[1, 64]]), _v(t2u[:], 64, [[1, 64]]))
                nc.gpsimd.tensor_copy(_v(t2u[:], 33 * 64, [[1, 64]]), _v(t2u[:], 32 * 64, [[1, 64]]))
                for p, (o1, a1, o2, a2) in enumerate([(0, 0.25, 64, 0.75), (64, 0.75, 128, 0.25)]):
                    outv = _v(up_acc[:], p * 64, [[128, 32], [1, 64]])
                    eng = nc.vector if p == 0 else nc.gpsimd
                    fma(eng, outv, _v(t2u[:], o1, [[64, 32], [1, 64]]), a1, outv)
                    fma(eng, outv, _v(t2u[:], o2, [[64, 32], [1, 64]]), a2, outv)

            def attn_small(fa, fT, N, isb, write_out, label):
                st = attn_A_start(fa, N, isb, label)
                for u in range(st["nunits"]):
                    attn_A_unit(st, u)
                attn_finish(st, fT, write_out)

            # ---------------- master schedule ----------------
            st0 = attn_A_start(f1a, N1, 0, "s1")
            st1 = attn_A_start(f1a, N1, 1, "s1")
            sts = [st0, st1, None, None, None, None, None, None]

            def diag_col(j):
                """x1 diag col (col 4) for q-tile j (sb j//4, sub j%4)."""
                return sts[j // 4]["x1"][j % 4][:, 4:5]

            # A-unit u (even cols of chunks 2u,2u+1) runnable after conv 2u+1
            asched = {1: [(st0, 0)], 2: [(st1, 0)], 3: [(st0, 1), (st1, 1)],
                      5: [(st0, 2), (st1, 2)], 7: [(st0, 3)]}
            for r in range(8):
                cp = pp.tile([C, 512], dt.float32, tag="b")
                for dy in range(3):   # pairs (dy,0)+(dy,1) on XA
                    rhs = _v(XA[:], (8 * r + dy) * PAD, [[PAD, 8], [1, W]])
                    nc.tensor.matmul(cp[:], wt16[:, dy * C:(dy + 1) * C], rhs,
                                     start=(dy == 0), stop=False)
                rhsB = _v(XB[:], (8 * r) * PAD + 2, [[PAD, 8], [1, W]])
                nc.tensor.matmul(cp[:], wt16[:, 3 * C:4 * C], rhsB,
                                 start=False, stop=False)
                rhsS = _v(XB[0:C, :], (8 * r + 2) * PAD + 2, [[PAD, 8], [1, W]])
                nc.tensor.matmul(cp[:], wt16[0:C, 4 * C:5 * C], rhsS,
                                 start=False, stop=True)
                nc.vector.tensor_copy(f1a[0:C, r * 512:(r + 1) * 512], cp[:])
                for st, u in asched.get(r, []):
                    attn_A_unit(st, u)
                if r >= 2 and sts[r] is None:
                    sts[r] = attn_A_start(f1a, N1, r, "s1")
                ft_convert(f1a, fT1, 4 * r, 4, diag_into=diag_col)
            attn_A_unit(st1, 3)
            st2, st3, st4, st5, st6, st7 = sts[2:]

            def fill_units(st):
                return [(lambda st=st, u=u: attn_A_unit(st, u))
                        for u in range(st["nunits"])]

            emit_pools2()
            attn_finish(st0, fT1, w1, filler=fill_units(st2))
            attn_finish(st1, fT1, w1, filler=fill_units(st3))
            emit_pools4()
            ft_convert(f2a, fT2, 0, 8)
            attn_finish(st2, fT1, w1, filler=fill_units(st4))
            attn_small(f2a, fT2, 1024, 0, w2, "s2")
            attn_finish(st3, fT1, w1, filler=fill_units(st5))
            attn_small(f2a, fT2, 1024, 1, w2, "s2")
            ft_convert(f4a, fT4, 0, 2)
            attn_finish(st4, fT1, w1, filler=fill_units(st6))
            attn_small(f4a, fT4, 256, 0, w4, "s4")
            emit_up4()
            attn_finish(st5, fT1, w1, filler=fill_units(st7))
            emit_up2()
            attn_finish(st6, fT1, w1)
            # last superblock: final up_acc add + most of the output DMA
            # overlap its B/C window.
            nc.vector.tensor_tensor(out_acc[:, 0:3584], out_acc[:, 0:3584],
                                    up_acc[:, 0:3584], op=ALU.add)
            nc.sync.dma_start(out_d.ap()[:, 0:3584], out_acc[:, 0:3584])
            attn_finish(st7, fT1, w1)
            nc.gpsimd.tensor_tensor(out_acc[:, 3584:N1], out_acc[:, 3584:N1],
                                    up_acc[:, 3584:N1], op=ALU.add)
            nc.sync.dma_start(out_d.ap()[:, 3584:N1], out_acc[:, 3584:N1])

    nc.compile()
    return nc


def _prep_inputs(x, W_std):
    lap = np.array([[0., 1., 0.], [1., -4., 1.], [0., 1., 0.]], dtype=np.float32)
    Wl = W_std.astype(np.float32) + lap[None, None] * np.eye(C, dtype=np.float32)[:, :, None, None]
    # tap-packed weights: [128, 5*C] fp16
    wt = np.zeros((128, 5 * C), dtype=np.float16)
    for dy in range(3):   # pairs (dy,0)+(dy,1)
        wt[0:C, dy * C:(dy + 1) * C] = Wl[:, :, dy, 0].T
        wt[C:128, dy * C:(dy + 1) * C] = Wl[:, :, dy, 1].T
    wt[0:C, 3 * C:4 * C] = Wl[:, :, 0, 2].T     # pair (0,2)+(1,2)
    wt[C:128, 3 * C:4 * C] = Wl[:, :, 1, 2].T
    wt[0:C, 4 * C:5 * C] = Wl[:, :, 2, 2].T     # single (2,2)
    B = x.shape[0]
    xps = np.zeros((B, C, PAD, PAD), dtype=np.float16)
    xps[:, :, 1:H + 1, 1:W + 1] = x.astype(np.float16)
    return xps.reshape(B, C, PAD * PAD), wt


def _run(x, W_std, trace=False):
    x = np.asarray(x)
    W_std = np.asarray(W_std)
    xps, wt = _prep_inputs(x, W_std)
    if "nc" not in _cache:
        _cache["nc"] = _build_nc()
    nc = _cache["nc"]
    in_maps = [{"xp": np.ascontiguousarray(xps[i]), "wt": wt} for i in range(x.shape[0])]
    ncores = min(NCORES, x.shape[0])
    res = run_bass_kernel_spmd(nc, in_maps, core_ids=list(range(ncores)), trace=trace)
    out = np.stack([res.results[i]["out"].reshape(C, H, W) for i in range(x.shape[0])])
    return out.astype(np.float32), res


def kernel(x, W_std):
    out, _ = _run(x, W_std, trace=False)
    return out


# revision 12
# speedup vs baseline: 1.1539x; 1.0513x over previous
"""Trainium2 Bass kernel for nn_BoundaryExtractionModule.

Data-parallel over batch: 8 samples -> 8 NeuronCores, one sample per core.

Per-core pipeline (channel-major layout [C, N] with C=64 on partitions):
  conv3x3(W_std)+depthwise-Laplacian  : tap-packed 5 matmuls per 512-col chunk
                                        (two shifted input copies stacked on
                                        128 partitions give K=128 tap pairs)
  3-scale non-local attention         : for each scale s in (4, 2, 1):
      A: approximate row-max c of the logits S = f^T f.  Scale 1 maxes the
         even columns (stride-2 A-matmuls, half cost) plus the diagonal
         |f_q|^2 (from fT squares); scales 2/4 max exactly.  A sigmoid
         in place of exp makes the softmax immune to the underestimate:
         sigmoid(x-d) = e^(x-d) for x <= c (0.3% at d=6) and saturates at 1
         for the rare rows whose true max was missed, so no overflow and
         the missed peak still dominates.  The shift d cancels in the ratio.
      B: recompute S^T with -(c+6) folded in via an augmented contraction
         row (K=65):  S'[m,q] = sum_k f_a[k,m] g_a[k,q],  g_a = [f; -(c+6)]
      sigmoid on ACT (PSUM -> fp16 SBUF) : E^T tiles
      C: PV matmul (fp16)               : G = [f;1] @ E^T, ones-column gives
                                          the softmax denominator
      D: normalize: reciprocal of the denominator row, gpsimd
         partition-broadcast, multiply, accumulate.
  bilinear x2/x4 upsample (half-pixel): stt-fused strided ops on DVE/Pool
  residual add + DMA out.
"""

import numpy as np

import concourse.bass as bass
import concourse.mybir as mybir
import concourse.tile as tile
from concourse import bacc
from concourse.bass_utils import run_bass_kernel_spmd
from concourse.masks import make_identity

dt = mybir.dt
AF = mybir.ActivationFunctionType
ALU = mybir.AluOpType
AX = mybir.AxisListType

C = 64
H = W = 64
N1 = H * W          # 4096
PAD = 66            # padded row length for conv
NCORES = 8
DSH = 6.0           # sigmoid shift

_cache = {}


def _v(ap, off, dims):
    """View of `ap` at free-offset `off` with free dims `dims` (keeps partition dim)."""
    return bass.AP(ap.tensor, ap.offset + off, [list(ap.ap[0])] + [list(d) for d in dims])


def _build_nc():
    nc = bacc.Bacc(None, target_bir_lowering=False)
    xp_d = nc.dram_tensor("xp", [C, PAD * PAD], dt.float16, kind="ExternalInput")
    wt_d = nc.dram_tensor("wt", [128, 5 * C], dt.float16, kind="ExternalInput")
    out_d = nc.dram_tensor("out", [C, N1], dt.float32, kind="ExternalOutput")

    with tile.TileContext(nc) as tc:
        with (
            tc.tile_pool(name="sb", bufs=1) as sb,
            tc.tile_pool(name="ga", bufs=4) as ga_pool,
            tc.tile_pool(name="et", bufs=6) as et_pool,
            tc.tile_pool(name="dd", bufs=4) as dd_pool,
            tc.tile_pool(name="aa", bufs=3, space="PSUM") as aa,
            tc.tile_pool(name="pp", bufs=2, space="PSUM") as pp,
            tc.tile_pool(name="gg", bufs=1, space="PSUM") as gg,
        ):
            # ---------------- inputs ----------------
            # XA: partitions 0:64 = x, 64:128 = x shifted +1 col (dx tap pairs)
            # XB: partitions 0:64 = x, 64:128 = x shifted +1 row
            XA = sb.tile([128, PAD * PAD], dt.float16)
            XB = sb.tile([128, PAD * PAD], dt.float16)
            head = 10 * PAD
            nc.sync.dma_start(XA[0:C, 0:head], xp_d.ap()[:, 0:head])
            nc.sync.dma_start(XA[C:128, 0:head], xp_d.ap()[:, 1:head + 1])
            nc.sync.dma_start(XB[0:C, 0:head], xp_d.ap()[:, 0:head])
            nc.sync.dma_start(XB[C:128, 0:head], xp_d.ap()[:, PAD:head + PAD])
            nc.sync.dma_start(XA[0:C, head:PAD * PAD], xp_d.ap()[:, head:PAD * PAD])
            nc.sync.dma_start(XA[C:128, head:PAD * PAD - 1], xp_d.ap()[:, head + 1:PAD * PAD])
            nc.sync.dma_start(XB[0:C, head:PAD * PAD], xp_d.ap()[:, head:PAD * PAD])
            nc.sync.dma_start(XB[C:128, head:PAD * PAD - PAD], xp_d.ap()[:, head + PAD:PAD * PAD])
            wt16 = sb.tile([128, 5 * C], dt.float16)
            nc.sync.dma_start(wt16[:], wt_d.ap())

            ident = sb.tile([128, 128], dt.float16)
            make_identity(nc, ident[:])

            out_acc = sb.tile([C, N1], dt.float32)
            # residual init: out_acc = x  (from the padded fp16 input)
            nc.gpsimd.tensor_copy(out_acc[:], _v(XA[0:C, :], PAD + 1, [[PAD, H], [1, W]]))

            # features (fp16) with augmented ones row
            f1a = sb.tile([C + 1, N1], dt.float16)
            nc.gpsimd.memset(f1a[C:C + 1, :], 1.0)

            # transposed features (C-pass lhsT, 65-row layout with ones col)
            fT1 = sb.tile([128, 32 * 65], dt.float16)
            fT2 = sb.tile([128, 8 * 65], dt.float16)
            fT4 = sb.tile([128, 2 * 65], dt.float16)
            for t, nt in ((fT1, 32), (fT2, 8), (fT4, 2)):
                nc.vector.memset(_v(t[:], C, [[65, nt], [1, 1]]), 1.0)
            dsq = sb.tile([128, C], dt.float16)   # fT square scratch

            def ft_convert(fa, fT, j0, nj, diag_into=None, on_act=False):
                """PE-transpose fa j-tiles into fT; optionally also compute
                the diagonal |f_q|^2 of tile j into diag_into(j) (an x1 col)."""
                for j in range(j0, j0 + nj, 2):
                    take = min(2, j0 + nj - j)
                    pt = aa.tile([128, 128], dt.float16, tag="a")
                    for u in range(take):
                        nc.tensor.transpose(pt[:, u * C:(u + 1) * C],
                                            fa[0:C, (j + u) * 128:(j + u + 1) * 128],
                                            ident[0:C, 0:C])
                    cp_eng = nc.scalar.copy if on_act else nc.vector.tensor_copy
                    cp_eng(_v(fT[:], j * 65, [[65, take], [1, C]]),
                           _v(pt[:], 0, [[C, take], [1, C]]))
                    if diag_into is not None:
                        for u in range(take):
                            ftv = fT[:, (j + u) * 65:(j + u) * 65 + C]
                            nc.gpsimd.tensor_tensor(dsq[:], ftv, ftv, op=ALU.mult)
                            nc.vector.reduce_sum(diag_into(j + u), dsq[:], axis=AX.X)

            # ---------------- A-pass: row-max candidates ----------------
            # scale 1: stride-2 even columns (4 tiles/sub) + diag col in x1.
            # scales 2/4: exact.  x2 = -(max over x1 cols) - DSH.
            def attn_A_start(fa, N, isb, label):
                q0 = isb * 512
                Q = min(512, N - q0)
                nsub = Q // 128
                st = dict(fa=fa, N=N, isb=isb, q0=q0, Q=Q, nsub=nsub,
                          nunits=4 if N > 1024 else (2 if N == 1024 else 1),
                          x1=[sb.tile([128, 6], dt.float32, name=f"x1_{label}_{isb}_{s}")
                              for s in range(nsub)],
                          x2=[sb.tile([128, 1], dt.float16, name=f"x2_{label}_{isb}_{s}")
                              for s in range(nsub)])
                return st

            def attn_A_unit(st, u):
                fa, q0, N = st["fa"], st["q0"], st["N"]
                for sub in range(st["nsub"]):
                    lhsA = fa[0:C, q0 + sub * 128: q0 + (sub + 1) * 128]
                    at = aa.tile([128, 512], dt.float32, tag="a")
                    if N > 1024:
                        # even columns of original chunk pair (2u, 2u+1)
                        rhs = _v(fa[0:C, :], u * 1024, [[2, 512]])
                        nc.tensor.matmul(at[:], lhsA, rhs, start=True, stop=True)
                        nc.vector.reduce_max(st["x1"][sub][:, u:u + 1], at[:],
                                             axis=AX.X)
                    else:
                        ln = min(512, N - u * 512)
                        nc.tensor.matmul(at[:, 0:ln], lhsA,
                                         fa[0:C, u * 512:u * 512 + ln],
                                         start=True, stop=True)
                        nc.vector.reduce_max(st["x1"][sub][:, u:u + 1], at[:, 0:ln],
                                             axis=AX.X)

            def attn_A_final(st):
                ncol = st["nunits"] + (1 if st["N"] > 1024 else 0)  # + diag col
                for sub in range(st["nsub"]):
                    x1 = st["x1"][sub]
                    m = dd_pool.tile([128, 1], dt.float32, tag="m")
                    nc.vector.reduce_max(m[:], x1[:, 0:ncol], axis=AX.X)
                    nc.vector.tensor_scalar(st["x2"][sub][:], m[:], -1.0, -DSH,
                                            op0=ALU.mult, op1=ALU.add)

            # ---------------- B + sigmoid + C + normalize ----------------
            def attn_finish(st, fT, write_out, filler=()):
                fa, N, isb = st["fa"], st["N"], st["isb"]
                q0, Q, nsub = st["q0"], st["Q"], st["nsub"]
                NT = N // 128
                attn_A_final(st)
                ga = ga_pool.tile([C + 1, 512], dt.float16, tag="ga")
                nc.gpsimd.tensor_copy(ga[0:C, 0:Q], fa[0:C, q0:q0 + Q])
                ptb = aa.tile([1, 512], dt.float16, tag="a")
                for sub in range(nsub):
                    # PE-transpose -(c+6) [128,1] -> [1,128] into the bias row
                    nc.tensor.transpose(ptb[:, sub * 128:(sub + 1) * 128],
                                        st["x2"][sub][:], ident[:])
                nc.vector.tensor_copy(ga[C:C + 1, 0:Q], ptb[:, 0:Q])
                G = gg.tile([C + 1, 512], dt.float32, tag="g")
                groups = [list(range(g, min(g + 2, NT))) for g in range(0, NT, 2)]
                filler = list(filler)
                fill_at = {int(i * len(groups) / len(filler)): i for i in range(len(filler))} if filler else {}
                for gi, grp in enumerate(groups):
                    if gi in fill_at:
                        filler[fill_at[gi]]()
                    bt = pp.tile([128, 1024], dt.float32, tag="b")
                    et = et_pool.tile([128, 1024], dt.float16, tag="et")
                    for jj, j in enumerate(grp):
                        nc.tensor.matmul(bt[:, jj * 512: jj * 512 + Q],
                                         fa[:, j * 128:(j + 1) * 128], ga[:, 0:Q],
                                         start=True, stop=True)
                    if Q == 512:
                        nc.scalar.activation(et[:], bt[:], AF.Sigmoid)
                    else:
                        for jj in range(len(grp)):
                            nc.scalar.activation(et[:, jj * 512:jj * 512 + Q],
                                                 bt[:, jj * 512:jj * 512 + Q],
                                                 AF.Sigmoid)
                    for jj, j in enumerate(grp):
                        nc.tensor.matmul(G[:, 0:Q], fT[:, j * 65:(j + 1) * 65],
                                         et[:, jj * 512:jj * 512 + Q],
                                         start=(gi == 0 and jj == 0),
                                         stop=(j == NT - 1))
                # --- D: normalize ---
                Gs = dd_pool.tile([C + 1, 512], dt.float32, tag="gs")
                nc.vector.tensor_copy(Gs[:, 0:Q], G[:, 0:Q])
                linv = dd_pool.tile([1, 512], dt.float32, tag="linv")
                nc.vector.reciprocal(linv[:, 0:Q], Gs[C:C + 1, 0:Q])
                lrep = dd_pool.tile([C, 512], dt.float32, tag="lrep")
                nc.gpsimd.partition_broadcast(lrep[:, 0:Q], linv[0:1, 0:Q])
                write_out(isb, q0, Q, Gs, lrep)

            def w1(isb, q0, Q, Gs, lrep):
                tmp = dd_pool.tile([C, 512], dt.float32, tag="tmp")
                nc.gpsimd.tensor_tensor(tmp[:, 0:Q], Gs[0:C, 0:Q], lrep[:, 0:Q], op=ALU.mult)
                nc.gpsimd.tensor_tensor(out_acc[:, q0:q0 + Q], out_acc[:, q0:q0 + Q],
                                        tmp[:, 0:Q], op=ALU.add)

            att2p = sb.tile([C, 34 * 34], dt.float32)   # scale-2 attn out, 1-px padded
            att4p = sb.tile([C, 18 * 18], dt.float32)   # scale-4 attn out, 1-px padded
            up_acc = sb.tile([C, N1], dt.float32)       # upsampled x2+x4 sum

            def w2(isb, q0, Q, Gs, lrep):
                r0 = isb * 16
                view = _v(att2p[:], (1 + r0) * 34 + 1, [[34, 16], [1, 32]])
                nc.gpsimd.tensor_tensor(view, Gs[0:C, 0:Q], lrep[:, 0:Q], op=ALU.mult)

            def w4(isb, q0, Q, Gs, lrep):
                view = _v(att4p[:], 18 + 1, [[18, 16], [1, 16]])
                nc.gpsimd.tensor_tensor(view, Gs[0:C, 0:Q], lrep[:, 0:Q], op=ALU.mult)

            # ---------------- pool emitters (DVE, fp16) ----------------
            f2raw = sb.tile([C, 1024], dt.float16)
            f2a = sb.tile([C + 1, 1024], dt.float16)
            f4a = sb.tile([C + 1, 256], dt.float16)

            def emit_pools2():
                f1 = f1a[0:C, :]
                t2w = sb.tile([C, 2048], dt.float16)
                nc.vector.tensor_tensor(t2w[:], _v(f1, 0, [[2, 2048]]), _v(f1, 1, [[2, 2048]]), op=ALU.add)
                nc.vector.tensor_tensor(f2raw[:], _v(t2w[:], 0, [[64, 32], [1, 32]]),
                                        _v(t2w[:], 32, [[64, 32], [1, 32]]), op=ALU.add)
                nc.vector.tensor_scalar_mul(f2a[0:C, :], f2raw[:], 0.25)
                nc.vector.memset(f2a[C:C + 1, :], 1.0)

            def emit_pools4():
                t4w = sb.tile([C, 512], dt.float16)
                nc.vector.tensor_tensor(t4w[:], _v(f2raw[:], 0, [[2, 512]]), _v(f2raw[:], 1, [[2, 512]]), op=ALU.add)
                f4raw = sb.tile([C, 256], dt.float16)
                nc.vector.tensor_tensor(f4raw[:], _v(t4w[:], 0, [[32, 16], [1, 16]]),
                                        _v(t4w[:], 16, [[32, 16], [1, 16]]), op=ALU.add)
                nc.vector.tensor_scalar_mul(f4a[0:C, :], f4raw[:], 1.0 / 16.0)
                nc.vector.memset(f4a[C:C + 1, :], 1.0)

            # ---------------- upsample emitters (stt-fused) ----------------
            ups_scr = sb.tile([C, 2048], dt.float32)   # upsample scratch

            def fma(eng, outv, in0, a, in1):
                """outv = in0*a + in1 (stt on DVE; mul+add pair on Pool)."""
                if eng is nc.vector:
                    eng.scalar_tensor_tensor(outv, in0, a, in1,
                                             op0=ALU.mult, op1=ALU.add)
                else:
                    sz = 1
                    for _, n in (in0.ap[1:] if len(in0.ap) > 1 else []):
                        sz *= n
                    scr = _v(ups_scr[:], 0, [[1, sz]])
                    eng.tensor_scalar_mul(scr, in0, a)
                    eng.tensor_tensor(outv, scr, in1, op=ALU.add)

            def emit_up4():
                p4 = att4p[:]
                nc.gpsimd.tensor_copy(_v(p4, 18, [[18, 16]]), _v(p4, 19, [[18, 16]]))
                nc.gpsimd.tensor_copy(_v(p4, 18 + 17, [[18, 16]]), _v(p4, 18 + 16, [[18, 16]]))
                nc.gpsimd.tensor_copy(_v(p4, 0, [[1, 18]]), _v(p4, 18, [[1, 18]]))
                nc.gpsimd.tensor_copy(_v(p4, 17 * 18, [[1, 18]]), _v(p4, 16 * 18, [[1, 18]]))
                t4u = sb.tile([C, 18 * 64], dt.float32)
                pre58 = sb.tile([C, 256], dt.float32)   # 0.625 * center
                pre78 = sb.tile([C, 256], dt.float32)   # 0.875 * center
                ctr = _v(p4, 18 + 1, [[18, 16], [1, 16]])
                nc.vector.tensor_scalar_mul(pre58[:], ctr, 0.625)
                nc.vector.tensor_scalar_mul(pre78[:], ctr, 0.875)
                lft = _v(p4, 18 + 0, [[18, 16], [1, 16]])
                rgt = _v(p4, 18 + 2, [[18, 16], [1, 16]])
                for p, (nb, a, pre) in enumerate([(lft, 0.375, pre58), (lft, 0.125, pre78),
                                                  (rgt, 0.125, pre78), (rgt, 0.375, pre58)]):
                    outv = _v(t4u[:], 64 + p, [[64, 16], [4, 16]])
                    fma(nc.vector, outv, nb, a, pre[:])
                nc.gpsimd.tensor_copy(_v(t4u[:], 0, [[1, 64]]), _v(t4u[:], 64, [[1, 64]]))
                nc.gpsimd.tensor_copy(_v(t4u[:], 17 * 64, [[1, 64]]), _v(t4u[:], 16 * 64, [[1, 64]]))
                for p, (o1, a1, o2, a2) in enumerate([(0, 0.375, 64, 0.625), (0, 0.125, 64, 0.875),
                                                      (64, 0.875, 128, 0.125), (64, 0.625, 128, 0.375)]):
                    outv = _v(up_acc[:], p * 64, [[256, 16], [1, 64]])
                    nc.vector.tensor_scalar_mul(outv, _v(t4u[:], o1, [[64, 16], [1, 64]]), a1)
                    fma(nc.vector, outv, _v(t4u[:], o2, [[64, 16], [1, 64]]), a2, outv)

            def emit_up2():
                p2 = att2p[:]
                nc.gpsimd.tensor_copy(_v(p2, 34, [[34, 32]]), _v(p2, 35, [[34, 32]]))
                nc.gpsimd.tensor_copy(_v(p2, 34 + 33, [[34, 32]]), _v(p2, 34 + 32, [[34, 32]]))
                nc.gpsimd.tensor_copy(_v(p2, 0, [[1, 34]]), _v(p2, 34, [[1, 34]]))
                nc.gpsimd.tensor_copy(_v(p2, 33 * 34, [[1, 34]]), _v(p2, 32 * 34, [[1, 34]]))
                t2u = sb.tile([C, 34 * 64], dt.float32)
                pre34 = sb.tile([C, 1024], dt.float32)  # 0.75 * center
                ctr2 = _v(p2, 34 + 1, [[34, 32], [1, 32]])
                nc.vector.tensor_scalar_mul(pre34[:], ctr2, 0.75)
                lft2 = _v(p2, 34 + 0, [[34, 32], [1, 32]])
                rgt2 = _v(p2, 34 + 2, [[34, 32], [1, 32]])
                for p, nb in enumerate([lft2, rgt2]):
                    outv = _v(t2u[:], 64 + p, [[64, 32], [2, 32]])
                    fma(nc.vector, outv, nb, 0.25, pre34[:])
                nc.gpsimd.tensor_copy(_v(t2u[:], 0, [[1, 64]]), _v(t2u[:], 64, [[1, 64]]))
                nc.gpsimd.tensor_copy(_v(t2u[:], 33 * 64, [[1, 64]]), _v(t2u[:], 32 * 64, [[1, 64]]))
                for p, (o1, a1, o2, a2) in enumerate([(0, 0.25, 64, 0.75), (64, 0.75, 128, 0.25)]):
                    outv = _v(up_acc[:], p * 64, [[128, 32], [1, 64]])
                    fma(nc.vector, outv, _v(t2u[:], o1, [[64, 32], [1, 64]]), a1, outv)
                    fma(nc.vector, outv, _v(t2u[:], o2, [[64, 32], [1, 64]]), a2, outv)

            def attn_small(fa, fT, N, isb, write_out, label):
                st = attn_A_start(fa, N, isb, label)
                for u in range(st["nunits"]):
                    attn_A_unit(st, u)
                attn_finish(st, fT, write_out)

            # ---------------- master schedule ----------------
            st0 = attn_A_start(f1a, N1, 0, "s1")
            st1 = attn_A_start(f1a, N1, 1, "s1")
            sts = [st0, st1, None, None, None, None, None, None]

            def diag_col(j):
                """x1 diag col (col 4) for q-tile j (sb j//4, sub j%4)."""
                return sts[j // 4]["x1"][j % 4][:, 4:5]

            # A-unit u (even cols of chunks 2u,2u+1) runnable after conv 2u+1
            asched = {1: [(st0, 0)], 2: [(st1, 0)], 3: [(st0, 1), (st1, 1)],
                      5: [(st0, 2), (st1, 2)], 7: [(st0, 3)]}
            for r in range(8):
                cp = pp.tile([C, 512], dt.float32, tag="b")
                for dy in range(3):   # pairs (dy,0)+(dy,1) on XA
                    rhs = _v(XA[:], (8 * r + dy) * PAD, [[PAD, 8], [1, W]])
                    nc.tensor.matmul(cp[:], wt16[:, dy * C:(dy + 1) * C], rhs,
                                     start=(dy == 0), stop=False)
                rhsB = _v(XB[:], (8 * r) * PAD + 2, [[PAD, 8], [1, W]])
                nc.tensor.matmul(cp[:], wt16[:, 3 * C:4 * C], rhsB,
                                 start=False, stop=False)
                rhsS = _v(XB[0:C, :], (8 * r + 2) * PAD + 2, [[PAD, 8], [1, W]])
                nc.tensor.matmul(cp[:], wt16[0:C, 4 * C:5 * C], rhsS,
                                 start=False, stop=True)
                nc.scalar.copy(f1a[0:C, r * 512:(r + 1) * 512], cp[:])
                for st, u in asched.get(r, []):
                    attn_A_unit(st, u)
                if r >= 2 and sts[r] is None:
                    sts[r] = attn_A_start(f1a, N1, r, "s1")
                ft_convert(f1a, fT1, 4 * r, 4, diag_into=diag_col, on_act=True)
            attn_A_unit(st1, 3)
            st2, st3, st4, st5, st6, st7 = sts[2:]

            def fill_units(st):
                return [(lambda st=st, u=u: attn_A_unit(st, u))
                        for u in range(st["nunits"])]

            emit_pools2()
            attn_finish(st0, fT1, w1, filler=fill_units(st2))
            attn_finish(st1, fT1, w1, filler=fill_units(st3))
            emit_pools4()
            ft_convert(f2a, fT2, 0, 8)
            attn_finish(st2, fT1, w1, filler=fill_units(st4))
            attn_small(f2a, fT2, 1024, 0, w2, "s2")
            attn_finish(st3, fT1, w1, filler=fill_units(st5))
            attn_small(f2a, fT2, 1024, 1, w2, "s2")
            ft_convert(f4a, fT4, 0, 2)
            attn_finish(st4, fT1, w1, filler=fill_units(st6))
            attn_small(f4a, fT4, 256, 0, w4, "s4")
            emit_up4()
            attn_finish(st5, fT1, w1, filler=fill_units(st7))
            emit_up2()
            attn_finish(st6, fT1, w1)
            # last superblock: final up_acc add + most of the output DMA
            # overlap its B/C window.
            nc.gpsimd.tensor_tensor(out_acc[:, 0:3584], out_acc[:, 0:3584],
                                    up_acc[:, 0:3584], op=ALU.add)
            nc.sync.dma_start(out_d.ap()[:, 0:3584], out_acc[:, 0:3584])
            attn_finish(st7, fT1, w1)
            nc.gpsimd.tensor_tensor(out_acc[:, 3584:N1], out_acc[:, 3584:N1],
                                    up_acc[:, 3584:N1], op=ALU.add)
            nc.sync.dma_start(out_d.ap()[:, 3584:N1], out_acc[:, 3584:N1])

    nc.compile()
    return nc


def _prep_inputs(x, W_std):
    lap = np.array([[0., 1., 0.], [1., -4., 1.], [0., 1., 0.]], dtype=np.float32)
    Wl = W_std.astype(np.float32) + lap[None, None] * np.eye(C, dtype=np.float32)[:, :, None, None]
    # tap-packed weights: [128, 5*C] fp16
    wt = np.zeros((128, 5 * C), dtype=np.float16)
    for dy in range(3):   # pairs (dy,0)+(dy,1)
        wt[0:C, dy * C:(dy + 1) * C] = Wl[:, :, dy, 0].T
        wt[C:128, dy * C:(dy + 1) * C] = Wl[:, :, dy, 1].T
    wt[0:C, 3 * C:4 * C] = Wl[:, :, 0, 2].T     # pair (0,2)+(1,2)
    wt[C:128, 3 * C:4 * C] = Wl[:, :, 1, 2].T
    wt[0:C, 4 * C:5 * C] = Wl[:, :, 2, 2].T     # single (2,2)
    B = x.shape[0]
    xps = np.zeros((B, C, PAD, PAD), dtype=np.float16)
    xps[:, :, 1:H + 1, 1:W + 1] = x.astype(np.float16)
    return xps.reshape(B, C, PAD * PAD), wt


def _run(x, W_std, trace=False):
    x = np.asarray(x)
    W_std = np.asarray(W_std)
    xps, wt = _prep_inputs(x, W_std)
    if "nc" not in _cache:
        _cache["nc"] = _build_nc()
    nc = _cache["nc"]
    in_maps = [{"xp": np.ascontiguousarray(xps[i]), "wt": wt} for i in range(x.shape[0])]
    ncores = min(NCORES, x.shape[0])
    res = run_bass_kernel_spmd(nc, in_maps, core_ids=list(range(ncores)), trace=trace)
    out = np.stack([res.results[i]["out"].reshape(C, H, W) for i in range(x.shape[0])])
    return out.astype(np.float32), res


def kernel(x, W_std):
    out, _ = _run(x, W_std, trace=False)
    return out


# revision 13
# speedup vs baseline: 1.2477x; 1.0813x over previous
"""Trainium2 Bass kernel for nn_BoundaryExtractionModule.

Data-parallel over batch: 8 samples -> 8 NeuronCores, one sample per core.

Per-core pipeline (channel-major layout [C, N] with C=64 on partitions):
  conv3x3(W_std)+depthwise-Laplacian  : tap-packed 5 matmuls per 512-col chunk
                                        (two shifted input copies stacked on
                                        128 partitions give K=128 tap pairs)
  3-scale non-local attention         : for each scale s in (4, 2, 1):
      A: approximate row-max c of the logits S = f^T f.  Scale 1 maxes the
         even columns (stride-2 A-matmuls, half cost) plus the diagonal
         |f_q|^2 (from fT squares); scales 2/4 max exactly.  A sigmoid
         in place of exp makes the softmax immune to the underestimate:
         sigmoid(x-d) = e^(x-d) for x <= c (0.3% at d=6) and saturates at 1
         for the rare rows whose true max was missed, so no overflow and
         the missed peak still dominates.  The shift d cancels in the ratio.
      B: recompute S^T with -(c+6) folded in via an augmented contraction
         row (K=65):  S'[m,q] = sum_k f_a[k,m] g_a[k,q],  g_a = [f; -(c+6)]
      sigmoid on ACT (PSUM -> fp16 SBUF) : E^T tiles
      C: PV matmul (fp16)               : G = [f;1] @ E^T, ones-column gives
                                          the softmax denominator
      D: normalize: reciprocal of the denominator row, gpsimd
         partition-broadcast, multiply, accumulate.
  bilinear x2/x4 upsample (half-pixel): stt-fused strided ops on DVE/Pool
  residual add + DMA out.
"""

import numpy as np

import concourse.bass as bass
import concourse.mybir as mybir
import concourse.tile as tile
from concourse import bacc
from concourse.bass_utils import run_bass_kernel_spmd
from concourse.masks import make_identity

dt = mybir.dt
AF = mybir.ActivationFunctionType
ALU = mybir.AluOpType
AX = mybir.AxisListType

C = 64
H = W = 64
N1 = H * W          # 4096
PAD = 66            # padded row length for conv
NCORES = 8
DSH = 6.0           # sigmoid shift

_cache = {}


def _v(ap, off, dims):
    """View of `ap` at free-offset `off` with free dims `dims` (keeps partition dim)."""
    return bass.AP(ap.tensor, ap.offset + off, [list(ap.ap[0])] + [list(d) for d in dims])


def _build_nc():
    nc = bacc.Bacc(None, target_bir_lowering=False)
    xp_d = nc.dram_tensor("xp", [C, PAD * PAD], dt.float16, kind="ExternalInput")
    wt_d = nc.dram_tensor("wt", [128, 6 * C], dt.float16, kind="ExternalInput")
    out_d = nc.dram_tensor("out", [C, N1], dt.float32, kind="ExternalOutput")

    with tile.TileContext(nc) as tc:
        with (
            tc.tile_pool(name="sb", bufs=1) as sb,
            tc.tile_pool(name="ga", bufs=4) as ga_pool,
            tc.tile_pool(name="et", bufs=6) as et_pool,
            tc.tile_pool(name="dd", bufs=4) as dd_pool,
            tc.tile_pool(name="aa", bufs=3, space="PSUM") as aa,
            tc.tile_pool(name="pp", bufs=2, space="PSUM") as pp,
            tc.tile_pool(name="gg", bufs=1, space="PSUM") as gg,
        ):
            # ---------------- inputs ----------------
            # XA: partitions 0:64 = x, 64:128 = x shifted +1 col (dx tap pairs)
            # XB: partitions 0:64 = x, 64:128 = x shifted +1 row
            XA = sb.tile([128, PAD * PAD], dt.float16)
            wt16 = sb.tile([128, 6 * C], dt.float16)
            nc.sync.dma_start(wt16[:], wt_d.ap())
            head = 10 * PAD
            nc.sync.dma_start(XA[0:C, 0:head], xp_d.ap()[:, 0:head])
            nc.sync.dma_start(XA[C:128, 0:head], xp_d.ap()[:, 1:head + 1])
            nc.sync.dma_start(XA[0:C, head:PAD * PAD], xp_d.ap()[:, head:PAD * PAD])
            nc.sync.dma_start(XA[C:128, head:PAD * PAD - 1], xp_d.ap()[:, head + 1:PAD * PAD])

            ident = sb.tile([128, 128], dt.float16)
            make_identity(nc, ident[:])

            out_acc = sb.tile([C, N1], dt.float32)
            # residual init: out_acc = x  (from the padded fp16 input)
            nc.gpsimd.tensor_copy(out_acc[:], _v(XA[0:C, :], PAD + 1, [[PAD, H], [1, W]]))

            # features (fp16) with augmented ones row
            f1a = sb.tile([C + 1, N1], dt.float16)
            nc.gpsimd.memset(f1a[C:C + 1, :], 1.0)

            # transposed features (C-pass lhsT, 65-row layout with ones col)
            fT1 = sb.tile([128, 32 * 65], dt.float16)
            fT2 = sb.tile([128, 8 * 65], dt.float16)
            fT4 = sb.tile([128, 2 * 65], dt.float16)
            for t, nt in ((fT1, 32), (fT2, 8), (fT4, 2)):
                nc.vector.memset(_v(t[:], C, [[65, nt], [1, 1]]), 1.0)
            dsq = sb.tile([128, C], dt.float16)   # fT square scratch

            def ft_convert(fa, fT, j0, nj, diag_into=None, on_act=False):
                """PE-transpose fa j-tiles into fT; optionally also compute
                the diagonal |f_q|^2 of tile j into diag_into(j) (an x1 col)."""
                for j in range(j0, j0 + nj, 2):
                    take = min(2, j0 + nj - j)
                    pt = aa.tile([128, 128], dt.float16, tag="a")
                    for u in range(take):
                        nc.tensor.transpose(pt[:, u * C:(u + 1) * C],
                                            fa[0:C, (j + u) * 128:(j + u + 1) * 128],
                                            ident[0:C, 0:C])
                    cp_eng = nc.scalar.copy if on_act else nc.vector.tensor_copy
                    cp_eng(_v(fT[:], j * 65, [[65, take], [1, C]]),
                           _v(pt[:], 0, [[C, take], [1, C]]))
                    if diag_into is not None:
                        for u in range(take):
                            ftv = fT[:, (j + u) * 65:(j + u) * 65 + C]
                            nc.gpsimd.tensor_tensor(dsq[:], ftv, ftv, op=ALU.mult)
                            nc.vector.reduce_sum(diag_into(j + u), dsq[:], axis=AX.X)

            # ---------------- A-pass: row-max candidates ----------------
            # scale 1: stride-2 even columns (4 tiles/sub) + diag col in x1.
            # scales 2/4: exact.  x2 = -(max over x1 cols) - DSH.
            def attn_A_start(fa, N, isb, label):
                q0 = isb * 512
                Q = min(512, N - q0)
                nsub = Q // 128
                st = dict(fa=fa, N=N, isb=isb, q0=q0, Q=Q, nsub=nsub,
                          nunits=4 if N > 1024 else (2 if N == 1024 else 1),
                          x1=[sb.tile([128, 6], dt.float32, name=f"x1_{label}_{isb}_{s}")
                              for s in range(nsub)],
                          x2=[sb.tile([128, 1], dt.float16, name=f"x2_{label}_{isb}_{s}")
                              for s in range(nsub)])
                return st

            def attn_A_unit(st, u):
                fa, q0, N = st["fa"], st["q0"], st["N"]
                for sub in range(st["nsub"]):
                    lhsA = fa[0:C, q0 + sub * 128: q0 + (sub + 1) * 128]
                    at = aa.tile([128, 512], dt.float32, tag="a")
                    if N > 1024:
                        # even columns of original chunk pair (2u, 2u+1)
                        rhs = _v(fa[0:C, :], u * 1024, [[2, 512]])
                        nc.tensor.matmul(at[:], lhsA, rhs, start=True, stop=True)
                        nc.vector.reduce_max(st["x1"][sub][:, u:u + 1], at[:],
                                             axis=AX.X)
                    else:
                        ln = min(512, N - u * 512)
                        nc.tensor.matmul(at[:, 0:ln], lhsA,
                                         fa[0:C, u * 512:u * 512 + ln],
                                         start=True, stop=True)
                        nc.vector.reduce_max(st["x1"][sub][:, u:u + 1], at[:, 0:ln],
                                             axis=AX.X)

            def attn_A_final(st):
                ncol = st["nunits"] + (1 if st["N"] > 1024 else 0)  # + diag col
                for sub in range(st["nsub"]):
                    x1 = st["x1"][sub]
                    m = dd_pool.tile([128, 1], dt.float32, tag="m")
                    nc.vector.reduce_max(m[:], x1[:, 0:ncol], axis=AX.X)
                    nc.vector.tensor_scalar(st["x2"][sub][:], m[:], -1.0, -DSH,
                                            op0=ALU.mult, op1=ALU.add)

            # ---------------- B + sigmoid + C + normalize ----------------
            def attn_finish(st, fT, write_out, filler=()):
                fa, N, isb = st["fa"], st["N"], st["isb"]
                q0, Q, nsub = st["q0"], st["Q"], st["nsub"]
                NT = N // 128
                attn_A_final(st)
                ga = ga_pool.tile([C + 1, 512], dt.float16, tag="ga")
                nc.gpsimd.tensor_copy(ga[0:C, 0:Q], fa[0:C, q0:q0 + Q])
                ptb = aa.tile([1, 512], dt.float16, tag="a")
                for sub in range(nsub):
                    # PE-transpose -(c+6) [128,1] -> [1,128] into the bias row
                    nc.tensor.transpose(ptb[:, sub * 128:(sub + 1) * 128],
                                        st["x2"][sub][:], ident[:])
                nc.vector.tensor_copy(ga[C:C + 1, 0:Q], ptb[:, 0:Q])
                G = gg.tile([C + 1, 512], dt.float32, tag="g")
                groups = [list(range(g, min(g + 2, NT))) for g in range(0, NT, 2)]
                filler = list(filler)
                fill_at = {int(i * len(groups) / len(filler)): i for i in range(len(filler))} if filler else {}
                for gi, grp in enumerate(groups):
                    if gi in fill_at:
                        filler[fill_at[gi]]()
                    bt = pp.tile([128, 1024], dt.float32, tag="b")
                    et = et_pool.tile([128, 1024], dt.float16, tag="et")
                    for jj, j in enumerate(grp):
                        nc.tensor.matmul(bt[:, jj * 512: jj * 512 + Q],
                                         fa[:, j * 128:(j + 1) * 128], ga[:, 0:Q],
                                         start=True, stop=True)
                    if Q == 512:
                        nc.scalar.activation(et[:], bt[:], AF.Sigmoid)
                    else:
                        for jj in range(len(grp)):
                            nc.scalar.activation(et[:, jj * 512:jj * 512 + Q],
                                                 bt[:, jj * 512:jj * 512 + Q],
                                                 AF.Sigmoid)
                    for jj, j in enumerate(grp):
                        nc.tensor.matmul(G[:, 0:Q], fT[:, j * 65:(j + 1) * 65],
                                         et[:, jj * 512:jj * 512 + Q],
                                         start=(gi == 0 and jj == 0),
                                         stop=(j == NT - 1))
                # --- D: normalize ---
                Gs = dd_pool.tile([C + 1, 512], dt.float32, tag="gs")
                nc.vector.tensor_copy(Gs[:, 0:Q], G[:, 0:Q])
                linv = dd_pool.tile([1, 512], dt.float32, tag="linv")
                nc.vector.reciprocal(linv[:, 0:Q], Gs[C:C + 1, 0:Q])
                lrep = dd_pool.tile([C, 512], dt.float32, tag="lrep")
                nc.gpsimd.partition_broadcast(lrep[:, 0:Q], linv[0:1, 0:Q])
                write_out(isb, q0, Q, Gs, lrep)

            def w1(isb, q0, Q, Gs, lrep):
                tmp = dd_pool.tile([C, 512], dt.float32, tag="tmp")
                nc.gpsimd.tensor_tensor(tmp[:, 0:Q], Gs[0:C, 0:Q], lrep[:, 0:Q], op=ALU.mult)
                nc.gpsimd.tensor_tensor(out_acc[:, q0:q0 + Q], out_acc[:, q0:q0 + Q],
                                        tmp[:, 0:Q], op=ALU.add)

            att2p = sb.tile([C, 34 * 34], dt.float32)   # scale-2 attn out, 1-px padded
            att4p = sb.tile([C, 18 * 18], dt.float32)   # scale-4 attn out, 1-px padded
            up_acc = sb.tile([C, N1], dt.float32)       # upsampled x2+x4 sum

            def w2(isb, q0, Q, Gs, lrep):
                r0 = isb * 16
                view = _v(att2p[:], (1 + r0) * 34 + 1, [[34, 16], [1, 32]])
                nc.gpsimd.tensor_tensor(view, Gs[0:C, 0:Q], lrep[:, 0:Q], op=ALU.mult)

            def w4(isb, q0, Q, Gs, lrep):
                view = _v(att4p[:], 18 + 1, [[18, 16], [1, 16]])
                nc.gpsimd.tensor_tensor(view, Gs[0:C, 0:Q], lrep[:, 0:Q], op=ALU.mult)

            # ---------------- pool emitters (DVE, fp16) ----------------
            f2raw = sb.tile([C, 1024], dt.float16)
            f2a = sb.tile([C + 1, 1024], dt.float16)
            f4a = sb.tile([C + 1, 256], dt.float16)

            def emit_pools2():
                f1 = f1a[0:C, :]
                t2w = sb.tile([C, 2048], dt.float16)
                nc.vector.tensor_tensor(t2w[:], _v(f1, 0, [[2, 2048]]), _v(f1, 1, [[2, 2048]]), op=ALU.add)
                nc.vector.tensor_tensor(f2raw[:], _v(t2w[:], 0, [[64, 32], [1, 32]]),
                                        _v(t2w[:], 32, [[64, 32], [1, 32]]), op=ALU.add)
                nc.vector.tensor_scalar_mul(f2a[0:C, :], f2raw[:], 0.25)
                nc.vector.memset(f2a[C:C + 1, :], 1.0)

            def emit_pools4():
                t4w = sb.tile([C, 512], dt.float16)
                nc.vector.tensor_tensor(t4w[:], _v(f2raw[:], 0, [[2, 512]]), _v(f2raw[:], 1, [[2, 512]]), op=ALU.add)
                f4raw = sb.tile([C, 256], dt.float16)
                nc.vector.tensor_tensor(f4raw[:], _v(t4w[:], 0, [[32, 16], [1, 16]]),
                                        _v(t4w[:], 16, [[32, 16], [1, 16]]), op=ALU.add)
                nc.vector.tensor_scalar_mul(f4a[0:C, :], f4raw[:], 1.0 / 16.0)
                nc.vector.memset(f4a[C:C + 1, :], 1.0)

            # ---------------- upsample emitters (stt-fused) ----------------
            ups_scr = sb.tile([C, 2048], dt.float32)   # upsample scratch

            def fma(eng, outv, in0, a, in1):
                """outv = in0*a + in1 (stt on DVE; mul+add pair on Pool)."""
                if eng is nc.vector:
                    eng.scalar_tensor_tensor(outv, in0, a, in1,
                                             op0=ALU.mult, op1=ALU.add)
                else:
                    sz = 1
                    for _, n in (in0.ap[1:] if len(in0.ap) > 1 else []):
                        sz *= n
                    scr = _v(ups_scr[:], 0, [[1, sz]])
                    eng.tensor_scalar_mul(scr, in0, a)
                    eng.tensor_tensor(outv, scr, in1, op=ALU.add)

            def emit_up4():
                thunks = []
                emit = thunks.append
                p4 = att4p[:]
                def _edges4():
                    nc.gpsimd.tensor_copy(_v(p4, 18, [[18, 16]]), _v(p4, 19, [[18, 16]]))
                    nc.gpsimd.tensor_copy(_v(p4, 18 + 17, [[18, 16]]), _v(p4, 18 + 16, [[18, 16]]))
                    nc.gpsimd.tensor_copy(_v(p4, 0, [[1, 18]]), _v(p4, 18, [[1, 18]]))
                    nc.gpsimd.tensor_copy(_v(p4, 17 * 18, [[1, 18]]), _v(p4, 16 * 18, [[1, 18]]))
                emit(_edges4)
                t4u = sb.tile([C, 18 * 64], dt.float32)
                pre58 = sb.tile([C, 256], dt.float32)   # 0.625 * center
                pre78 = sb.tile([C, 256], dt.float32)   # 0.875 * center
                ctr = _v(p4, 18 + 1, [[18, 16], [1, 16]])
                lft = _v(p4, 18 + 0, [[18, 16], [1, 16]])
                rgt = _v(p4, 18 + 2, [[18, 16], [1, 16]])
                def _pre4():
                    nc.vector.tensor_scalar_mul(pre58[:], ctr, 0.625)
                    nc.vector.tensor_scalar_mul(pre78[:], ctr, 0.875)
                emit(_pre4)
                for p, (nb, a, pre) in enumerate([(lft, 0.375, pre58), (lft, 0.125, pre78),
                                                  (rgt, 0.125, pre78), (rgt, 0.375, pre58)]):
                    outv = _v(t4u[:], 64 + p, [[64, 16], [4, 16]])
                    emit(lambda outv=outv, nb=nb, a=a, pre=pre:
                         fma(nc.vector, outv, nb, a, pre[:]))
                def _edges4b():
                    nc.gpsimd.tensor_copy(_v(t4u[:], 0, [[1, 64]]), _v(t4u[:], 64, [[1, 64]]))
                    nc.gpsimd.tensor_copy(_v(t4u[:], 17 * 64, [[1, 64]]), _v(t4u[:], 16 * 64, [[1, 64]]))
                emit(_edges4b)
                for p, (o1, a1, o2, a2) in enumerate([(0, 0.375, 64, 0.625), (0, 0.125, 64, 0.875),
                                                      (64, 0.875, 128, 0.125), (64, 0.625, 128, 0.375)]):
                    def _h4(p=p, o1=o1, a1=a1, o2=o2, a2=a2):
                        outv = _v(up_acc[:], p * 64, [[256, 16], [1, 64]])
                        nc.vector.tensor_scalar_mul(outv, _v(t4u[:], o1, [[64, 16], [1, 64]]), a1)
                        fma(nc.vector, outv, _v(t4u[:], o2, [[64, 16], [1, 64]]), a2, outv)
                    emit(_h4)
                return thunks

            def emit_up2():
                thunks = []
                emit = thunks.append
                p2 = att2p[:]
                def _edges2():
                    nc.gpsimd.tensor_copy(_v(p2, 34, [[34, 32]]), _v(p2, 35, [[34, 32]]))
                    nc.gpsimd.tensor_copy(_v(p2, 34 + 33, [[34, 32]]), _v(p2, 34 + 32, [[34, 32]]))
                    nc.gpsimd.tensor_copy(_v(p2, 0, [[1, 34]]), _v(p2, 34, [[1, 34]]))
                    nc.gpsimd.tensor_copy(_v(p2, 33 * 34, [[1, 34]]), _v(p2, 32 * 34, [[1, 34]]))
                emit(_edges2)
                t2u = sb.tile([C, 34 * 64], dt.float32)
                pre34 = sb.tile([C, 1024], dt.float32)  # 0.75 * center
                ctr2 = _v(p2, 34 + 1, [[34, 32], [1, 32]])
                lft2 = _v(p2, 34 + 0, [[34, 32], [1, 32]])
                rgt2 = _v(p2, 34 + 2, [[34, 32], [1, 32]])
                emit(lambda: nc.vector.tensor_scalar_mul(pre34[:], ctr2, 0.75))
                for p, nb in enumerate([lft2, rgt2]):
                    outv = _v(t2u[:], 64 + p, [[64, 32], [2, 32]])
                    emit(lambda outv=outv, nb=nb: fma(nc.vector, outv, nb, 0.25, pre34[:]))
                def _edges2b():
                    nc.gpsimd.tensor_copy(_v(t2u[:], 0, [[1, 64]]), _v(t2u[:], 64, [[1, 64]]))
                    nc.gpsimd.tensor_copy(_v(t2u[:], 33 * 64, [[1, 64]]), _v(t2u[:], 32 * 64, [[1, 64]]))
                emit(_edges2b)
                for p, (o1, a1, o2, a2) in enumerate([(0, 0.25, 64, 0.75), (64, 0.75, 128, 0.25)]):
                    def _h2a(p=p, o1=o1, a1=a1):
                        outv = _v(up_acc[:], p * 64, [[128, 32], [1, 64]])
                        fma(nc.vector, outv, _v(t2u[:], o1, [[64, 32], [1, 64]]), a1, outv)
                    def _h2b(p=p, o2=o2, a2=a2):
                        outv = _v(up_acc[:], p * 64, [[128, 32], [1, 64]])
                        fma(nc.vector, outv, _v(t2u[:], o2, [[64, 32], [1, 64]]), a2, outv)
                    emit(_h2a)
                    emit(_h2b)
                return thunks

            def attn_small(fa, fT, N, isb, write_out, label):
                st = attn_A_start(fa, N, isb, label)
                for u in range(st["nunits"]):
                    attn_A_unit(st, u)
                attn_finish(st, fT, write_out)

            # ---------------- master schedule ----------------
            st0 = attn_A_start(f1a, N1, 0, "s1")
            st1 = attn_A_start(f1a, N1, 1, "s1")
            sts = [st0, st1, None, None, None, None, None, None]

            def diag_col(j):
                """x1 diag col (col 4) for q-tile j (sb j//4, sub j%4)."""
                return sts[j // 4]["x1"][j % 4][:, 4:5]

            # A-unit u (even cols of chunks 2u,2u+1) runnable after conv 2u+1
            asched = {1: [(st0, 0)], 2: [(st1, 0)], 3: [(st0, 1), (st1, 1)],
                      5: [(st0, 2), (st1, 2)], 7: [(st0, 3)]}
            for r in range(8):
                cp = pp.tile([C, 512], dt.float32, tag="b")
                for dy in range(3):   # pairs (dy,0)+(dy,1) on XA
                    rhs = _v(XA[:], (8 * r + dy) * PAD, [[PAD, 8], [1, W]])
                    nc.tensor.matmul(cp[:], wt16[:, dy * C:(dy + 1) * C], rhs,
                                     start=(dy == 0), stop=False)
                for dy in range(3):   # singles (dy,2) on XA lower half
                    rhsS = _v(XA[0:C, :], (8 * r + dy) * PAD + 2, [[PAD, 8], [1, W]])
                    nc.tensor.matmul(cp[:], wt16[0:C, (3 + dy) * C:(4 + dy) * C], rhsS,
                                     start=False, stop=(dy == 2))
                nc.scalar.copy(f1a[0:C, r * 512:(r + 1) * 512], cp[:])
                for st, u in asched.get(r, []):
                    attn_A_unit(st, u)
                if r >= 2 and sts[r] is None:
                    sts[r] = attn_A_start(f1a, N1, r, "s1")
                ft_convert(f1a, fT1, 4 * r, 4, diag_into=diag_col, on_act=True)
            attn_A_unit(st1, 3)
            st2, st3, st4, st5, st6, st7 = sts[2:]

            def fill_units(st):
                return [(lambda st=st, u=u: attn_A_unit(st, u))
                        for u in range(st["nunits"])]

            emit_pools2()
            attn_finish(st0, fT1, w1, filler=fill_units(st2))
            attn_finish(st1, fT1, w1, filler=fill_units(st3))
            emit_pools4()
            ft_convert(f2a, fT2, 0, 8)
            attn_finish(st2, fT1, w1, filler=fill_units(st4))
            attn_small(f2a, fT2, 1024, 0, w2, "s2")
            attn_finish(st3, fT1, w1, filler=fill_units(st5))
            attn_small(f2a, fT2, 1024, 1, w2, "s2")
            ft_convert(f4a, fT4, 0, 2)
            attn_finish(st4, fT1, w1, filler=fill_units(st6))
            attn_small(f4a, fT4, 256, 0, w4, "s4")
            attn_finish(st5, fT1, w1, filler=fill_units(st7) + emit_up4())
            attn_finish(st6, fT1, w1, filler=emit_up2())
            # last superblock: final up_acc add + most of the output DMA
            # overlap its B/C window.
            nc.gpsimd.tensor_tensor(out_acc[:, 0:3584], out_acc[:, 0:3584],
                                    up_acc[:, 0:3584], op=ALU.add)
            nc.sync.dma_start(out_d.ap()[:, 0:3584], out_acc[:, 0:3584])
            attn_finish(st7, fT1, w1)
            nc.gpsimd.tensor_tensor(out_acc[:, 3584:N1], out_acc[:, 3584:N1],
                                    up_acc[:, 3584:N1], op=ALU.add)
            nc.sync.dma_start(out_d.ap()[:, 3584:N1], out_acc[:, 3584:N1])

    nc.compile()
    return nc


def _prep_inputs(x, W_std):
    lap = np.array([[0., 1., 0.], [1., -4., 1.], [0., 1., 0.]], dtype=np.float32)
    Wl = W_std.astype(np.float32) + lap[None, None] * np.eye(C, dtype=np.float32)[:, :, None, None]
    # tap-packed weights: [128, 6*C] fp16
    wt = np.zeros((128, 6 * C), dtype=np.float16)
    for dy in range(3):   # pairs (dy,0)+(dy,1)
        wt[0:C, dy * C:(dy + 1) * C] = Wl[:, :, dy, 0].T
        wt[C:128, dy * C:(dy + 1) * C] = Wl[:, :, dy, 1].T
    for dy in range(3):   # singles (dy,2)
        wt[0:C, (3 + dy) * C:(4 + dy) * C] = Wl[:, :, dy, 2].T
    B = x.shape[0]
    xps = np.zeros((B, C, PAD, PAD), dtype=np.float16)
    xps[:, :, 1:H + 1, 1:W + 1] = x.astype(np.float16)
    return xps.reshape(B, C, PAD * PAD), wt


def _run(x, W_std, trace=False):
    x = np.asarray(x)
    W_std = np.asarray(W_std)
    xps, wt = _prep_inputs(x, W_std)
    if "nc" not in _cache:
        _cache["nc"] = _build_nc()
    nc = _cache["nc"]
    in_maps = [{"xp": np.ascontiguousarray(xps[i]), "wt": wt} for i in range(x.shape[0])]
    ncores = min(NCORES, x.shape[0])
    res = run_bass_kernel_spmd(nc, in_maps, core_ids=list(range(ncores)), trace=trace)
    out = np.stack([res.results[i]["out"].reshape(C, H, W) for i in range(x.shape[0])])
    return out.astype(np.float32), res


def kernel(x, W_std):
    out, _ = _run(x, W_std, trace=False)
    return out
